# revision 1
# baseline (speedup 1.0000x reference)
"""MoE LoRA delta kernel for Trainium2 (8 NeuronCores, data-parallel over tokens).

Computation (per token t):
    logits = x @ router_w.T                      [T, 4]
    gates  = top2-softmax(logits)                [T, 4]  (exactly 2 nonzero)
    mid    = x @ A_all.T                         [T, 64]   A_all[(e,r), d]
    delta  = (mid * expand(gates) * 4.0) @ B_all [T, D]    B_all[(e,r), d]

Kernel strategy per core (T_c = 1024 tokens):
  - W = concat([A_all, router_w]) -> [68, D]; host passes W.T [D, 68] so the
    stationary operand loads directly.  mm1 computes [68, T] = W @ x.T with
    fp32 accumulation; rows 64:68 are the router logits (exact-enough fp32 so
    the top-2 expert selection matches the fp32 reference).
  - x.T tiles produced on-chip with PE transpose-mode matmuls (fp32, exact).
  - Gating runs with tokens on partitions (small PE transposes of the logits),
    all in fp32: g_e = 1{t_e >= m2} * sigmoid(2*t_e - m2), t = l - max(l).
  - Gates expanded to (e,r) rows and scaled by 4.0 with a tiny selection
    matmul, then mid is scaled elementwise and fed to mm2 against B_all.
"""

import os
import sys

for _p in ("/opt/trn_rl_repo", "/root/.axon_site/_ro/trn_rl_repo"):
    if os.path.isdir(_p) and _p not in sys.path:
        sys.path.insert(0, _p)

import numpy as np
from contextlib import ExitStack

import concourse.bass as bass
import concourse.bacc as bacc
import concourse.mybir as mybir
import concourse.tile as tile

N_CORES = 8
B_, S, D = 4, 2048, 3840
T_FULL = B_ * S                 # 8192
T_C = T_FULL // N_CORES         # 1024 tokens per core
E, R = 4, 16
ER = E * R                      # 64
M_W = ER + E                    # 68 = A rows + router rows
LORA_SCALE = 16.0 / np.sqrt(16.0)   # 4.0

GROUP = 256                     # tokens per mm1 group
TPG = GROUP // 128              # token tiles per group (2)
N_GROUPS = T_C // GROUP         # 4
D_CHUNKS = D // 128             # 30
MM2_CHUNKS = [(i * 512, min(512, D - i * 512)) for i in range((D + 511) // 512)]

F32 = mybir.dt.float32
F32R = mybir.dt.float32r

# Matmul input dtype mode: "f32" (safe) or "f32r" (fast, possibly lower precision)
MM_MODE = os.environ.get("MOE_MM_MODE", "f32")


def _mm_ap(ap):
    if MM_MODE == "f32r":
        return ap.bitcast(F32R)
    return ap


def _tp_ap(ap):
    # transpose-mode operands stay fp32 (exact data movement)
    return ap


def build_kernel(tc: tile.TileContext, out_d, x_d, wt_d, b_d, sel_d, id_d):
    nc = tc.nc
    with ExitStack() as ctx:
        const_pool = ctx.enter_context(tc.tile_pool(name="const", bufs=1))
        xin_pool = ctx.enter_context(tc.tile_pool(name="xin", bufs=2))
        xt_pool = ctx.enter_context(tc.tile_pool(name="xt", bufs=2))
        mid_pool = ctx.enter_context(tc.tile_pool(name="mid", bufs=2))
        g_pool = ctx.enter_context(tc.tile_pool(name="gate", bufs=2))
        dout_pool = ctx.enter_context(tc.tile_pool(name="dout", bufs=2))
        ps_tp = ctx.enter_context(
            tc.tile_pool(name="ps_tp", bufs=3, space=bass.MemorySpace.PSUM))
        ps_mm1 = ctx.enter_context(
            tc.tile_pool(name="ps_mm1", bufs=2, space=bass.MemorySpace.PSUM))
        ps_g = ctx.enter_context(
            tc.tile_pool(name="ps_g", bufs=1, space=bass.MemorySpace.PSUM))
        ps_mm2 = ctx.enter_context(
            tc.tile_pool(name="ps_mm2", bufs=2, space=bass.MemorySpace.PSUM))

        # ---- constants / weights ----
        wt_sb = const_pool.tile([128, D_CHUNKS, M_W], F32, tag="wt")
        nc.sync.dma_start(wt_sb[:], wt_d.rearrange("(c p) m -> p c m", p=128))
        b_sb = const_pool.tile([ER, D], F32, tag="ball")
        nc.sync.dma_start(b_sb[:], b_d[:])
        sel_sb = const_pool.tile([E, ER], F32, tag="sel")
        nc.sync.dma_start(sel_sb[:], sel_d[:])
        id_sb = const_pool.tile([128, 128], F32, tag="ident")
        nc.sync.dma_start(id_sb[:], id_d[:])

        copy_engines = [nc.vector, nc.scalar]
        cp_i = 0

        for g in range(N_GROUPS):
            tok_g = g * GROUP
            # ---- load + transpose x for this group ----
            xt_sb = xt_pool.tile([128, D_CHUNKS, GROUP], F32, tag="xt")
            for tl in range(TPG):
                tok0 = tok_g + tl * 128
                x_sb = xin_pool.tile([128, D], F32, tag="xin")
                nc.sync.dma_start(x_sb[:], x_d[tok0:tok0 + 128, :])
                for c0 in range(0, D_CHUNKS, 2):
                    tp_ps = ps_tp.tile([128, 2, 128], F32, tag="tp")
                    for cc in range(2):
                        c = c0 + cc
                        nc.tensor.transpose(
                            tp_ps[:, cc, :],
                            _tp_ap(x_sb[:, c * 128:(c + 1) * 128]),
                            _tp_ap(id_sb[:]),
                        )
                    eng = copy_engines[cp_i % 2]; cp_i += 1
                    if eng is nc.vector:
                        eng.tensor_copy(
                            xt_sb[:, c0:c0 + 2, tl * 128:(tl + 1) * 128], tp_ps[:])
                    else:
                        eng.copy(
                            xt_sb[:, c0:c0 + 2, tl * 128:(tl + 1) * 128], tp_ps[:])

            # ---- mm1: [68, GROUP] = W @ x.T (fp32 accumulation over D) ----
            mid_ps = ps_mm1.tile([M_W, GROUP], F32, tag="mm1")
            for c in range(D_CHUNKS):
                nc.tensor.matmul(
                    mid_ps[:],
                    _mm_ap(wt_sb[:, c, :]),
                    _mm_ap(xt_sb[:, c, :]),
                    start=(c == 0),
                    stop=(c == D_CHUNKS - 1),
                )

            # ---- gating (fp32, tokens on partitions) ----
            # copy logits rows (64:68) to SBUF so PE can transpose them
            lg_sb = g_pool.tile([M_W, GROUP], F32, tag="lg")
            nc.vector.tensor_copy(lg_sb[ER:M_W, :], mid_ps[ER:M_W, :])

            logT_ps = ps_g.tile([128, TPG, E], F32, tag="gps")
            for tl in range(TPG):
                nc.tensor.matmul(
                    logT_ps[:, tl, :],
                    lg_sb[ER:M_W, tl * 128:(tl + 1) * 128],
                    id_sb[ER:M_W, ER:M_W],
                    is_transpose=True,
                )

            gates_sb = g_pool.tile([128, TPG, E], F32, tag="gates")
            for tl in range(TPG):
                L = g_pool.tile([128, E], F32, tag="L")
                nc.vector.tensor_copy(L[:], logT_ps[:, tl, :])
                m1 = g_pool.tile([128, 1], F32, tag="m1")
                nc.vector.tensor_reduce(
                    m1[:], L[:], axis=mybir.AxisListType.X, op=mybir.AluOpType.max)
                tt = g_pool.tile([128, E], F32, tag="tt")
                nc.vector.tensor_scalar(
                    tt[:], L[:], m1[:], None, op0=mybir.AluOpType.subtract)
                z = g_pool.tile([128, E], F32, tag="z")
                nc.vector.tensor_scalar(
                    z[:], tt[:], 0.0, None, op0=mybir.AluOpType.is_equal)
                msk = g_pool.tile([128, E], F32, tag="msk")
                nc.vector.scalar_tensor_tensor(
                    msk[:], z[:], -1e30, tt[:],
                    op0=mybir.AluOpType.mult, op1=mybir.AluOpType.add)
                m2 = g_pool.tile([128, 1], F32, tag="m2")
                nc.vector.tensor_reduce(
                    m2[:], msk[:], axis=mybir.AxisListType.X, op=mybir.AluOpType.max)
                s2 = g_pool.tile([128, E], F32, tag="s2")
                nc.vector.tensor_scalar(
                    s2[:], tt[:], 2.0, m2[:],
                    op0=mybir.AluOpType.mult, op1=mybir.AluOpType.subtract)
                sg = g_pool.tile([128, E], F32, tag="sg")
                nc.scalar.activation(
                    sg[:], s2[:], mybir.ActivationFunctionType.Sigmoid)
                ge = g_pool.tile([128, E], F32, tag="ge")
                nc.vector.tensor_scalar(
                    ge[:], tt[:], m2[:], None, op0=mybir.AluOpType.is_ge)
                nc.vector.tensor_tensor(
                    gates_sb[:, tl, :], ge[:], sg[:], op=mybir.AluOpType.mult)

            # transpose gates back: [4, GROUP]
            gT_ps = ps_g.tile([E, GROUP], F32, tag="gps")
            for tl in range(TPG):
                nc.tensor.matmul(
                    gT_ps[:, tl * 128:(tl + 1) * 128],
                    gates_sb[:, tl, :],
                    id_sb[:],
                    is_transpose=True,
                )
            gT_sb = g_pool.tile([E, GROUP], F32, tag="gT")
            nc.vector.tensor_copy(gT_sb[:], gT_ps[:])

            # expand to (e,r) rows with the 4.0-scaled selection matrix
            gexp_ps = ps_g.tile([ER, GROUP], F32, tag="gps")
            nc.tensor.matmul(gexp_ps[:], sel_sb[:], gT_sb[:])
            gexp_sb = g_pool.tile([ER, GROUP], F32, tag="gexp")
            nc.scalar.copy(gexp_sb[:], gexp_ps[:])

            # scale mid by gates
            midTs = mid_pool.tile([ER, GROUP], F32, tag="midTs")
            nc.vector.tensor_tensor(
                midTs[:], mid_ps[0:ER, :], gexp_sb[:], op=mybir.AluOpType.mult)

            # ---- mm2: delta[t, d] = midTs.T @ B_all ----
            for tl in range(TPG):
                tok0 = tok_g + tl * 128
                dout_sb = dout_pool.tile([128, D], F32, tag="dout")
                for (d0, w) in MM2_CHUNKS:
                    mm2_ps = ps_mm2.tile([128, 512], F32, tag="mm2")
                    nc.tensor.matmul(
                        mm2_ps[:, 0:w],
                        _mm_ap(midTs[:, tl * 128:(tl + 1) * 128]),
                        _mm_ap(b_sb[:, d0:d0 + w]),
                    )
                    eng = copy_engines[cp_i % 2]; cp_i += 1
                    if eng is nc.vector:
                        eng.tensor_copy(dout_sb[:, d0:d0 + w], mm2_ps[:, 0:w])
                    else:
                        eng.copy(dout_sb[:, d0:d0 + w], mm2_ps[:, 0:w])
                nc.sync.dma_start(out_d[tok0:tok0 + 128, :], dout_sb[:])


_CACHED = {}


def _build_module():
    key = MM_MODE
    if key in _CACHED:
        return _CACHED[key]
    nc = bacc.Bacc("TRN2", target_bir_lowering=False, debug=False)
    x_d = nc.dram_tensor("x_in", [T_C, D], F32, kind="ExternalInput").ap()
    wt_d = nc.dram_tensor("wt_in", [D, M_W], F32, kind="ExternalInput").ap()
    b_d = nc.dram_tensor("ball_in", [ER, D], F32, kind="ExternalInput").ap()
    sel_d = nc.dram_tensor("sel_in", [E, ER], F32, kind="ExternalInput").ap()
    id_d = nc.dram_tensor("id_in", [128, 128], F32, kind="ExternalInput").ap()
    out_d = nc.dram_tensor("out", [T_C, D], F32, kind="ExternalOutput").ap()
    with tile.TileContext(nc) as tc:
        build_kernel(tc, out_d, x_d, wt_d, b_d, sel_d, id_d)
    nc.compile()
    _CACHED[key] = nc
    return nc


def _host_weights(router_w, A, B):
    W = np.concatenate([A.reshape(ER, D), router_w], axis=0).astype(np.float32)
    WT = np.ascontiguousarray(W.T)                                   # [D, 68]
    B_all = np.ascontiguousarray(
        B.transpose(0, 2, 1).reshape(ER, D)).astype(np.float32)      # [(e,r), d]
    sel = np.zeros((E, ER), np.float32)
    for e in range(E):
        sel[e, e * R:(e + 1) * R] = LORA_SCALE
    ident = np.eye(128, dtype=np.float32)
    return WT, B_all, sel, ident


def make_in_maps(x, router_w, A, B):
    flat = np.ascontiguousarray(np.asarray(x, np.float32).reshape(T_FULL, D))
    WT, B_all, sel, ident = _host_weights(
        np.asarray(router_w, np.float32),
        np.asarray(A, np.float32),
        np.asarray(B, np.float32))
    in_maps = []
    for i in range(N_CORES):
        in_maps.append({
            "x_in": flat[i * T_C:(i + 1) * T_C],
            "wt_in": WT,
            "ball_in": B_all,
            "sel_in": sel,
            "id_in": ident,
        })
    return in_maps


def kernel(x, router_w, A, B, _results_hook=None):
    from concourse.bass_utils import run_bass_kernel_spmd

    nc = _build_module()
    in_maps = make_in_maps(x, router_w, A, B)
    res = run_bass_kernel_spmd(nc, in_maps, core_ids=list(range(N_CORES)))
    if _results_hook is not None:
        _results_hook(res)
    out = np.concatenate([res.results[i]["out"] for i in range(N_CORES)], axis=0)
    return out.reshape(B_, S, D)


if __name__ == "__main__":
    rng = np.random.default_rng(0)
    x = rng.standard_normal((B_, S, D), dtype=np.float32)
    rw = (rng.standard_normal((E, D)) * 0.02).astype(np.float32)
    A = (rng.standard_normal((E, R, D)) * 0.02).astype(np.float32)
    Bm = (rng.standard_normal((E, D, R)) * 0.02).astype(np.float32)
    out = kernel(x, rw, A, Bm)
    print("out", out.shape, out.dtype, float(np.abs(out).max()))



# revision 3
# speedup vs baseline: 1.7119x; 1.7119x over previous
"""MoE LoRA delta kernel for Trainium2 (8 NeuronCores, data-parallel over tokens).

Computation (per token t):
    logits = x @ router_w.T                      [T, 4]
    gates  = top2-softmax(logits)                [T, 4]  (exactly 2 nonzero)
    mid    = x @ A_all.T                         [T, 64]   A_all[(e,r), d]
    delta  = (mid * expand(gates)) @ (4*B_all)   [T, D]    B_all[(e,r), d]

Kernel strategy per core (T_c = 1024 tokens, groups of 256):
  - x tiles are transposed on-chip (PE transpose, fp32 exact) into xt [D, T].
  - One fused fp32 matmul chain per 128-token tile computes token-major
    [128, 68] = x_tile @ [A_all.T | router_w.T]: cols 0:64 are mid, cols
    64:68 are the router logits (fp32-exact so the top-2 expert selection
    matches the fp32 reference).
  - Gating runs token-major in fp32: g_e = 1{t_e >= m2} * sigmoid(2*t_e - m2),
    t = l - max(l).  Gates scale mid in-place (per-expert per-partition
    scalar multiply) with fp16 output; the LoRA scale 4.0 is folded into B.
  - mid.T is restored with a tiny fp16 PE transpose, then mm2 (fp16) computes
    delta[t, d] = midT.T @ (4*B_all); results are written out as fp16 and
    upcast to fp32 on the host (max elementwise error ~1e-3 << 2e-2 gate).
"""

import os
import sys

for _p in ("/opt/trn_rl_repo", "/root/.axon_site/_ro/trn_rl_repo"):
    if os.path.isdir(_p) and _p not in sys.path:
        sys.path.insert(0, _p)

import numpy as np
from contextlib import ExitStack

import concourse.bass as bass
import concourse.bacc as bacc
import concourse.mybir as mybir
import concourse.tile as tile

N_CORES = 8
B_, S, D = 4, 2048, 3840
T_FULL = B_ * S                 # 8192
T_C = T_FULL // N_CORES         # 1024 tokens per core
E, R = 4, 16
ER = E * R                      # 64
M_W = ER + E                    # 68 = A rows + router rows
LORA_SCALE = 16.0 / np.sqrt(16.0)   # 4.0 (folded into B on the host)

GROUP = 256                     # tokens per group
TPG = GROUP // 128              # token tiles per group (2)
N_GROUPS = T_C // GROUP         # 4
D_CHUNKS = D // 128             # 30
TP_C = 3                        # chunks per transpose-psum tile
MM2_CHUNKS = [(i * 512, min(512, D - i * 512)) for i in range((D + 511) // 512)]

F32 = mybir.dt.float32
F16 = mybir.dt.float16


def build_kernel(tc: tile.TileContext, out_d, x_d, wt_d, b_d, id_d, id16_d):
    nc = tc.nc
    with ExitStack() as ctx:
        const_pool = ctx.enter_context(tc.tile_pool(name="const", bufs=1))
        xin_pool = ctx.enter_context(tc.tile_pool(name="xin", bufs=2))
        xt_pool = ctx.enter_context(tc.tile_pool(name="xt", bufs=2))
        mts_pool = ctx.enter_context(tc.tile_pool(name="mts", bufs=2))
        g_pool = ctx.enter_context(tc.tile_pool(name="gate", bufs=2))
        dout_pool = ctx.enter_context(tc.tile_pool(name="dout", bufs=2))
        ps_tp = ctx.enter_context(
            tc.tile_pool(name="ps_tp", bufs=3, space=bass.MemorySpace.PSUM))
        ps_mid = ctx.enter_context(
            tc.tile_pool(name="ps_mid", bufs=2, space=bass.MemorySpace.PSUM))
        ps_mt = ctx.enter_context(
            tc.tile_pool(name="ps_mt", bufs=1, space=bass.MemorySpace.PSUM))
        ps_mm2 = ctx.enter_context(
            tc.tile_pool(name="ps_mm2", bufs=2, space=bass.MemorySpace.PSUM))

        # ---- constants / weights ----
        wt_sb = const_pool.tile([128, D_CHUNKS, M_W], F32, tag="wt")
        nc.sync.dma_start(wt_sb[:], wt_d[:])
        b_sb = const_pool.tile([ER, D], F16, tag="ball")
        nc.sync.dma_start(b_sb[:], b_d[:])
        id_sb = const_pool.tile([128, 128], F32, tag="ident")
        nc.sync.dma_start(id_sb[:], id_d[:])
        id16_sb = const_pool.tile([128, 128], F16, tag="ident16")
        nc.sync.dma_start(id16_sb[:], id16_d[:])

        copy_engines = [nc.vector, nc.scalar]
        cp_state = {"i": 0}

        def copy_any(dst, src):
            eng = copy_engines[cp_state["i"] % 2]
            cp_state["i"] += 1
            if eng is nc.vector:
                eng.tensor_copy(dst, src)
            else:
                eng.copy(dst, src)

        def emit_loads(g):
            """DMA x tiles, PE-transpose them, copy to SBUF xt (fp32)."""
            tok_g = g * GROUP
            xt_sb = xt_pool.tile([128, D_CHUNKS, GROUP], F32, tag="xt")
            for tl in range(TPG):
                tok0 = tok_g + tl * 128
                x_sb = xin_pool.tile([128, D], F32, tag="xin")
                nc.sync.dma_start(x_sb[:], x_d[tok0:tok0 + 128, :])
                for c0 in range(0, D_CHUNKS, TP_C):
                    tp_ps = ps_tp.tile([128, TP_C, 128], F32, tag="tp")
                    for cc in range(TP_C):
                        c = c0 + cc
                        nc.tensor.transpose(
                            tp_ps[:, cc, :], x_sb[:, c * 128:(c + 1) * 128],
                            id_sb[:])
                    copy_any(
                        xt_sb[:, c0:c0 + TP_C, tl * 128:(tl + 1) * 128],
                        tp_ps[:])
            return xt_sb

        def emit_mm1(g, xt_sb):
            """Fused token-major matmul: [128, 68] = x_tile @ [A.T | rw.T]."""
            mid_ps = ps_mid.tile([128, TPG, M_W], F32, tag="mid")
            for tl in range(TPG):
                for c in range(D_CHUNKS):
                    nc.tensor.matmul(
                        mid_ps[:, tl, :],
                        xt_sb[:, c, tl * 128:(tl + 1) * 128],
                        wt_sb[:, c, :],
                        start=(c == 0),
                        stop=(c == D_CHUNKS - 1),
                    )
            return mid_ps

        def emit_gating(g, mid_ps):
            """Top-2 softmax gating, token-major, and gate-scaled fp16 mid."""
            mts_tm = []
            for tl in range(TPG):
                L = g_pool.tile([128, E], F32, tag="L")
                nc.vector.tensor_copy(L[:], mid_ps[:, tl, ER:M_W])
                m1 = g_pool.tile([128, 1], F32, tag="m1")
                nc.vector.tensor_reduce(
                    m1[:], L[:], axis=mybir.AxisListType.X,
                    op=mybir.AluOpType.max)
                tt = g_pool.tile([128, E], F32, tag="tt")
                nc.vector.tensor_scalar(
                    tt[:], L[:], m1[:], None, op0=mybir.AluOpType.subtract)
                z = g_pool.tile([128, E], F32, tag="z")
                nc.vector.tensor_scalar(
                    z[:], tt[:], 0.0, None, op0=mybir.AluOpType.is_equal)
                msk = g_pool.tile([128, E], F32, tag="msk")
                nc.vector.scalar_tensor_tensor(
                    msk[:], z[:], -1e30, tt[:],
                    op0=mybir.AluOpType.mult, op1=mybir.AluOpType.add)
                m2 = g_pool.tile([128, 1], F32, tag="m2")
                nc.vector.tensor_reduce(
                    m2[:], msk[:], axis=mybir.AxisListType.X,
                    op=mybir.AluOpType.max)
                s2 = g_pool.tile([128, E], F32, tag="s2")
                nc.vector.tensor_scalar(
                    s2[:], tt[:], 2.0, m2[:],
                    op0=mybir.AluOpType.mult, op1=mybir.AluOpType.subtract)
                sg = g_pool.tile([128, E], F32, tag="sg")
                nc.scalar.activation(
                    sg[:], s2[:], mybir.ActivationFunctionType.Sigmoid)
                ge = g_pool.tile([128, E], F32, tag="ge")
                nc.vector.tensor_scalar(
                    ge[:], tt[:], m2[:], None, op0=mybir.AluOpType.is_ge)
                gates = g_pool.tile([128, E], F32, tag="gates")
                nc.vector.tensor_tensor(
                    gates[:], ge[:], sg[:], op=mybir.AluOpType.mult)

                # scale mid rows by the per-expert gate (per-partition scalar)
                mt = mts_pool.tile([128, ER], F16, tag="mtm")
                for e in range(E):
                    nc.vector.tensor_scalar(
                        mt[:, e * R:(e + 1) * R],
                        mid_ps[:, tl, e * R:(e + 1) * R],
                        gates[:, e:e + 1], None,
                        op0=mybir.AluOpType.mult)
                mts_tm.append(mt)
            return mts_tm

        def emit_tail(g, mts_tm):
            """Transpose gated mid back to [er, t] (fp16), mm2, write out."""
            tok_g = g * GROUP
            mt_ps = ps_mt.tile([ER, TPG, 128], F16, tag="mt")
            mts_sb = mts_pool.tile([ER, TPG, 128], F16, tag="mts")
            for tl in range(TPG):
                nc.tensor.transpose(
                    mt_ps[:, tl, :], mts_tm[tl][:], id16_sb[:])
                copy_any(mts_sb[:, tl, :], mt_ps[:, tl, :])
            for tl in range(TPG):
                tok0 = tok_g + tl * 128
                dout_sb = dout_pool.tile([128, D], F16, tag="dout")
                for (d0, w) in MM2_CHUNKS:
                    mm2_ps = ps_mm2.tile([128, 512], F32, tag="mm2")
                    nc.tensor.matmul(
                        mm2_ps[:, 0:w],
                        mts_sb[:, tl, :],
                        b_sb[:, d0:d0 + w],
                    )
                    copy_any(dout_sb[:, d0:d0 + w], mm2_ps[:, 0:w])
                nc.sync.dma_start(out_d[tok0:tok0 + 128, :], dout_sb[:])

        # Software pipeline: loads for group g+1 are emitted between group
        # g's gating (DVE) and its mm2, so the PE transposes of g+1 overlap
        # the gating chain and DMA stays saturated.
        xt_sb = emit_loads(0)
        for g in range(N_GROUPS):
            mid_ps = emit_mm1(g, xt_sb)
            mts_tm = emit_gating(g, mid_ps)
            if g + 1 < N_GROUPS:
                xt_sb = emit_loads(g + 1)
            emit_tail(g, mts_tm)


_CACHED = {}


def _build_module():
    key = "tm"
    if key in _CACHED:
        return _CACHED[key]
    nc = bacc.Bacc("TRN2", target_bir_lowering=False, debug=False)
    x_d = nc.dram_tensor("x_in", [T_C, D], F32, kind="ExternalInput").ap()
    wt_d = nc.dram_tensor(
        "wt_in", [128, D_CHUNKS, M_W], F32, kind="ExternalInput").ap()
    b_d = nc.dram_tensor("ball_in", [ER, D], F16, kind="ExternalInput").ap()
    id_d = nc.dram_tensor("id_in", [128, 128], F32, kind="ExternalInput").ap()
    id16_d = nc.dram_tensor(
        "id16_in", [128, 128], F16, kind="ExternalInput").ap()
    out_d = nc.dram_tensor("out", [T_C, D], F16, kind="ExternalOutput").ap()
    with tile.TileContext(nc) as tc:
        build_kernel(tc, out_d, x_d, wt_d, b_d, id_d, id16_d)
    nc.compile()
    _CACHED[key] = nc
    return nc


def _host_weights(router_w, A, B):
    W = np.concatenate([A.reshape(ER, D), router_w], axis=0).astype(np.float32)
    # [128, D_CHUNKS, 68]: element [p, c, m] = W.T[c*128 + p, m]
    WT = np.ascontiguousarray(
        W.T.reshape(D_CHUNKS, 128, M_W).transpose(1, 0, 2))
    B_all = np.ascontiguousarray(
        (B.transpose(0, 2, 1).reshape(ER, D) * LORA_SCALE)).astype(np.float16)
    ident = np.eye(128, dtype=np.float32)
    ident16 = np.eye(128, dtype=np.float16)
    return WT, B_all, ident, ident16


def make_in_maps(x, router_w, A, B):
    flat = np.ascontiguousarray(np.asarray(x, np.float32).reshape(T_FULL, D))
    WT, B_all, ident, ident16 = _host_weights(
        np.asarray(router_w, np.float32),
        np.asarray(A, np.float32),
        np.asarray(B, np.float32))
    in_maps = []
    for i in range(N_CORES):
        in_maps.append({
            "x_in": flat[i * T_C:(i + 1) * T_C],
            "wt_in": WT,
            "ball_in": B_all,
            "id_in": ident,
            "id16_in": ident16,
        })
    return in_maps


def kernel(x, router_w, A, B, _results_hook=None):
    from concourse.bass_utils import run_bass_kernel_spmd

    nc = _build_module()
    in_maps = make_in_maps(x, router_w, A, B)
    res = run_bass_kernel_spmd(nc, in_maps, core_ids=list(range(N_CORES)))
    if _results_hook is not None:
        _results_hook(res)
    out = np.concatenate(
        [np.asarray(res.results[i]["out"]) for i in range(N_CORES)], axis=0)
    return out.astype(np.float32).reshape(B_, S, D)


if __name__ == "__main__":
    rng = np.random.default_rng(0)
    x = rng.standard_normal((B_, S, D), dtype=np.float32)
    rw = (rng.standard_normal((E, D)) * 0.02).astype(np.float32)
    A = (rng.standard_normal((E, R, D)) * 0.02).astype(np.float32)
    Bm = (rng.standard_normal((E, D, R)) * 0.02).astype(np.float32)
    out = kernel(x, rw, A, Bm)
    print("out", out.shape, out.dtype, float(np.abs(out).max()))


# revision 7
# speedup vs baseline: 1.7340x; 1.0129x over previous
"""MoE LoRA delta kernel for Trainium2 (8 NeuronCores, data-parallel over tokens).

Computation (per token t):
    logits = x @ router_w.T                      [T, 4]
    gates  = top2-softmax(logits)                [T, 4]  (exactly 2 nonzero)
    mid    = x @ A_all.T                         [T, 64]   A_all[(e,r), d]
    delta  = (mid * expand(gates)) @ (4*B_all)   [T, D]    B_all[(e,r), d]

Kernel strategy per core (T_c = 1024 tokens, groups of 256):
  - x tiles are transposed on-chip (PE transpose, fp32 exact) into xt [D, T].
  - One fused fp32 matmul chain per 128-token tile computes token-major
    [128, 68] = x_tile @ [A_all.T | router_w.T]: cols 0:64 are mid, cols
    64:68 are the router logits (fp32-exact so the top-2 expert selection
    matches the fp32 reference).
  - Gating runs token-major in fp32: g_e = 1{t_e >= m2} * sigmoid(2*t_e - m2),
    t = l - max(l).  Gates scale mid in-place (per-expert per-partition
    scalar multiply) with fp16 output; the LoRA scale 4.0 is folded into B.
  - mid.T is restored with a tiny fp16 PE transpose, then mm2 (fp16) computes
    delta[t, d] = midT.T @ (4*B_all); results are written out as fp16 and
    upcast to fp32 on the host (max elementwise error ~1e-3 << 2e-2 gate).
"""

import os
import sys

for _p in ("/opt/trn_rl_repo", "/root/.axon_site/_ro/trn_rl_repo"):
    if os.path.isdir(_p) and _p not in sys.path:
        sys.path.insert(0, _p)

import numpy as np
from contextlib import ExitStack

import concourse.bass as bass
import concourse.bacc as bacc
import concourse.mybir as mybir
import concourse.tile as tile

N_CORES = 8
B_, S, D = 4, 2048, 3840
T_FULL = B_ * S                 # 8192
T_C = T_FULL // N_CORES         # 1024 tokens per core
E, R = 4, 16
ER = E * R                      # 64
M_W = ER + E                    # 68 = A rows + router rows
LORA_SCALE = 16.0 / np.sqrt(16.0)   # 4.0 (folded into B on the host)

GROUP = 256                     # tokens per group
TPG = GROUP // 128              # token tiles per group (2)
N_GROUPS = T_C // GROUP         # 4
D_CHUNKS = D // 128             # 30
TP_C = 3                        # chunks per transpose-psum tile
MM2_CHUNKS = [(i * 512, min(512, D - i * 512)) for i in range((D + 511) // 512)]

F32 = mybir.dt.float32
F16 = mybir.dt.float16


def build_kernel(tc: tile.TileContext, out_d, x_d, wt_d, b_d, id_d, id16_d):
    nc = tc.nc
    with ExitStack() as ctx:
        const_pool = ctx.enter_context(tc.tile_pool(name="const", bufs=1))
        xin_pool = ctx.enter_context(tc.tile_pool(name="xin", bufs=3))
        xt_pool = ctx.enter_context(tc.tile_pool(name="xt", bufs=2))
        mts_pool = ctx.enter_context(tc.tile_pool(name="mts", bufs=2))
        g_pool = ctx.enter_context(tc.tile_pool(name="gate", bufs=2))
        dout_pool = ctx.enter_context(tc.tile_pool(name="dout", bufs=2))
        ps_tp = ctx.enter_context(
            tc.tile_pool(name="ps_tp", bufs=3, space=bass.MemorySpace.PSUM))
        ps_mid = ctx.enter_context(
            tc.tile_pool(name="ps_mid", bufs=2, space=bass.MemorySpace.PSUM))
        ps_mt = ctx.enter_context(
            tc.tile_pool(name="ps_mt", bufs=1, space=bass.MemorySpace.PSUM))
        ps_mm2 = ctx.enter_context(
            tc.tile_pool(name="ps_mm2", bufs=2, space=bass.MemorySpace.PSUM))

        # ---- constants / weights ----
        # identities first (needed by the first transposes); the big weight
        # loads are emitted after group 0's x tiles (emit_weights below) so
        # they stay off the startup critical path.
        id_sb = const_pool.tile([128, 128], F32, tag="ident")
        nc.sync.dma_start(id_sb[:], id_d[:])
        id16_sb = const_pool.tile([128, 128], F16, tag="ident16")
        nc.sync.dma_start(id16_sb[:], id16_d[:])
        wt_sb = const_pool.tile([128, D_CHUNKS, M_W], F32, tag="wt")
        b_sb = const_pool.tile([ER, D], F16, tag="ball")

        def emit_weights():
            nc.sync.dma_start(wt_sb[:], wt_d[:])
            nc.sync.dma_start(b_sb[:], b_d[:])

        copy_engines = [nc.vector, nc.scalar]
        cp_state = {"i": 0}

        def copy_any(dst, src):
            eng = copy_engines[cp_state["i"] % 2]
            cp_state["i"] += 1
            if eng is nc.vector:
                eng.tensor_copy(dst, src)
            else:
                eng.copy(dst, src)

        def emit_loads(g):
            """DMA x tiles, PE-transpose them, copy to SBUF xt (fp32)."""
            tok_g = g * GROUP
            xt_sb = xt_pool.tile([128, D_CHUNKS, GROUP], F32, tag="xt")
            for tl in range(TPG):
                tok0 = tok_g + tl * 128
                x_sb = xin_pool.tile([128, D], F32, tag="xin")
                nc.sync.dma_start(x_sb[:], x_d[tok0:tok0 + 128, :])
                for c0 in range(0, D_CHUNKS, TP_C):
                    tp_ps = ps_tp.tile([128, TP_C, 128], F32, tag="tp")
                    for cc in range(TP_C):
                        c = c0 + cc
                        nc.tensor.transpose(
                            tp_ps[:, cc, :], x_sb[:, c * 128:(c + 1) * 128],
                            id_sb[:])
                    copy_any(
                        xt_sb[:, c0:c0 + TP_C, tl * 128:(tl + 1) * 128],
                        tp_ps[:])
            return xt_sb

        def emit_mm1(g, xt_sb):
            """Fused token-major matmul: [128, 68] = x_tile @ [A.T | rw.T]."""
            mid_ps = ps_mid.tile([128, TPG, M_W], F32, tag="mid")
            for tl in range(TPG):
                for c in range(D_CHUNKS):
                    nc.tensor.matmul(
                        mid_ps[:, tl, :],
                        xt_sb[:, c, tl * 128:(tl + 1) * 128],
                        wt_sb[:, c, :],
                        start=(c == 0),
                        stop=(c == D_CHUNKS - 1),
                    )
            return mid_ps

        def emit_gating(g, mid_ps):
            """Top-2 softmax gating, token-major, and gate-scaled fp16 mid."""
            mts_tm = []
            for tl in range(TPG):
                L = g_pool.tile([128, E], F32, tag="L")
                nc.vector.tensor_copy(L[:], mid_ps[:, tl, ER:M_W])
                m1 = g_pool.tile([128, 1], F32, tag="m1")
                nc.vector.tensor_reduce(
                    m1[:], L[:], axis=mybir.AxisListType.X,
                    op=mybir.AluOpType.max)
                tt = g_pool.tile([128, E], F32, tag="tt")
                nc.vector.tensor_scalar(
                    tt[:], L[:], m1[:], None, op0=mybir.AluOpType.subtract)
                z = g_pool.tile([128, E], F32, tag="z")
                nc.vector.tensor_scalar(
                    z[:], tt[:], 0.0, None, op0=mybir.AluOpType.is_equal)
                msk = g_pool.tile([128, E], F32, tag="msk")
                nc.vector.scalar_tensor_tensor(
                    msk[:], z[:], -1e30, tt[:],
                    op0=mybir.AluOpType.mult, op1=mybir.AluOpType.add)
                m2 = g_pool.tile([128, 1], F32, tag="m2")
                nc.vector.tensor_reduce(
                    m2[:], msk[:], axis=mybir.AxisListType.X,
                    op=mybir.AluOpType.max)
                s2 = g_pool.tile([128, E], F32, tag="s2")
                nc.vector.tensor_scalar(
                    s2[:], tt[:], 2.0, m2[:],
                    op0=mybir.AluOpType.mult, op1=mybir.AluOpType.subtract)
                sg = g_pool.tile([128, E], F32, tag="sg")
                nc.scalar.activation(
                    sg[:], s2[:], mybir.ActivationFunctionType.Sigmoid)
                ge = g_pool.tile([128, E], F32, tag="ge")
                nc.vector.tensor_scalar(
                    ge[:], tt[:], m2[:], None, op0=mybir.AluOpType.is_ge)
                gates = g_pool.tile([128, E], F32, tag="gates")
                nc.vector.tensor_tensor(
                    gates[:], ge[:], sg[:], op=mybir.AluOpType.mult)

                # scale mid rows by the per-expert gate (per-partition scalar)
                mt = mts_pool.tile([128, ER], F16, tag="mtm")
                for e in range(E):
                    nc.vector.tensor_scalar(
                        mt[:, e * R:(e + 1) * R],
                        mid_ps[:, tl, e * R:(e + 1) * R],
                        gates[:, e:e + 1], None,
                        op0=mybir.AluOpType.mult)
                mts_tm.append(mt)
            return mts_tm

        def emit_tail(g, mts_tm):
            """Transpose gated mid back to [er, t] (fp16), mm2, write out."""
            tok_g = g * GROUP
            mt_ps = ps_mt.tile([ER, TPG, 128], F16, tag="mt")
            mts_sb = mts_pool.tile([ER, TPG, 128], F16, tag="mts")
            for tl in range(TPG):
                nc.tensor.transpose(
                    mt_ps[:, tl, :], mts_tm[tl][:], id16_sb[:])
                copy_any(mts_sb[:, tl, :], mt_ps[:, tl, :])
            for tl in range(TPG):
                tok0 = tok_g + tl * 128
                dout_sb = dout_pool.tile([128, D], F16, tag="dout")
                for (d0, w) in MM2_CHUNKS:
                    mm2_ps = ps_mm2.tile([128, 512], F32, tag="mm2")
                    nc.tensor.matmul(
                        mm2_ps[:, 0:w],
                        mts_sb[:, tl, :],
                        b_sb[:, d0:d0 + w],
                    )
                    copy_any(dout_sb[:, d0:d0 + w], mm2_ps[:, 0:w])
                # stores go out via the (otherwise idle) Pool engine's SWDGE
                # path so they never head-of-line-block the SP load queue
                nc.gpsimd.dma_start(out_d[tok0:tok0 + 128, :], dout_sb[:])

        # Software pipeline: loads for group g+1 are emitted between group
        # g's gating (DVE) and its mm2, so the PE transposes of g+1 overlap
        # the gating chain and DMA stays saturated.
        xt_sb = emit_loads(0)
        emit_weights()
        for g in range(N_GROUPS):
            mid_ps = emit_mm1(g, xt_sb)
            mts_tm = emit_gating(g, mid_ps)
            if g + 1 < N_GROUPS:
                xt_sb = emit_loads(g + 1)
            emit_tail(g, mts_tm)


_CACHED = {}


def _build_module():
    key = "tm"
    if key in _CACHED:
        return _CACHED[key]
    nc = bacc.Bacc("TRN2", target_bir_lowering=False, debug=False)
    x_d = nc.dram_tensor("x_in", [T_C, D], F32, kind="ExternalInput").ap()
    wt_d = nc.dram_tensor(
        "wt_in", [128, D_CHUNKS, M_W], F32, kind="ExternalInput").ap()
    b_d = nc.dram_tensor("ball_in", [ER, D], F16, kind="ExternalInput").ap()
    id_d = nc.dram_tensor("id_in", [128, 128], F32, kind="ExternalInput").ap()
    id16_d = nc.dram_tensor(
        "id16_in", [128, 128], F16, kind="ExternalInput").ap()
    out_d = nc.dram_tensor("out", [T_C, D], F16, kind="ExternalOutput").ap()
    with tile.TileContext(nc) as tc:
        build_kernel(tc, out_d, x_d, wt_d, b_d, id_d, id16_d)
    nc.compile()
    _CACHED[key] = nc
    return nc


def _host_weights(router_w, A, B):
    W = np.concatenate([A.reshape(ER, D), router_w], axis=0).astype(np.float32)
    # [128, D_CHUNKS, 68]: element [p, c, m] = W.T[c*128 + p, m]
    WT = np.ascontiguousarray(
        W.T.reshape(D_CHUNKS, 128, M_W).transpose(1, 0, 2))
    B_all = np.ascontiguousarray(
        (B.transpose(0, 2, 1).reshape(ER, D) * LORA_SCALE)).astype(np.float16)
    ident = np.eye(128, dtype=np.float32)
    ident16 = np.eye(128, dtype=np.float16)
    return WT, B_all, ident, ident16


def make_in_maps(x, router_w, A, B):
    flat = np.ascontiguousarray(np.asarray(x, np.float32).reshape(T_FULL, D))
    WT, B_all, ident, ident16 = _host_weights(
        np.asarray(router_w, np.float32),
        np.asarray(A, np.float32),
        np.asarray(B, np.float32))
    in_maps = []
    for i in range(N_CORES):
        in_maps.append({
            "x_in": flat[i * T_C:(i + 1) * T_C],
            "wt_in": WT,
            "ball_in": B_all,
            "id_in": ident,
            "id16_in": ident16,
        })
    return in_maps


def kernel(x, router_w, A, B, _results_hook=None):
    from concourse.bass_utils import run_bass_kernel_spmd

    nc = _build_module()
    in_maps = make_in_maps(x, router_w, A, B)
    res = run_bass_kernel_spmd(nc, in_maps, core_ids=list(range(N_CORES)))
    if _results_hook is not None:
        _results_hook(res)
    out = np.concatenate(
        [np.asarray(res.results[i]["out"]) for i in range(N_CORES)], axis=0)
    return out.astype(np.float32).reshape(B_, S, D)


if __name__ == "__main__":
    rng = np.random.default_rng(0)
    x = rng.standard_normal((B_, S, D), dtype=np.float32)
    rw = (rng.standard_normal((E, D)) * 0.02).astype(np.float32)
    A = (rng.standard_normal((E, R, D)) * 0.02).astype(np.float32)
    Bm = (rng.standard_normal((E, D, R)) * 0.02).astype(np.float32)
    out = kernel(x, rw, A, Bm)
    print("out", out.shape, out.dtype, float(np.abs(out).max()))


# revision 15
# speedup vs baseline: 1.7471x; 1.0076x over previous
"""MoE LoRA delta kernel for Trainium2 (8 NeuronCores, data-parallel over tokens).

Computation (per token t):
    logits = x @ router_w.T                      [T, 4]
    gates  = top2-softmax(logits)                [T, 4]  (exactly 2 nonzero)
    mid    = x @ A_all.T                         [T, 64]   A_all[(e,r), d]
    delta  = (mid * expand(gates)) @ (4*B_all)   [T, D]    B_all[(e,r), d]

Kernel strategy per core (T_c = 1024 tokens, groups of 256):
  - x tiles are transposed on-chip (PE transpose, fp32 exact) into xt [D, T].
  - One fused fp32 matmul chain per 128-token tile computes token-major
    [128, 68] = x_tile @ [A_all.T | router_w.T]: cols 0:64 are mid, cols
    64:68 are the router logits (fp32-exact so the top-2 expert selection
    matches the fp32 reference).
  - Gating runs token-major in fp32: g_e = 1{t_e >= m2} * sigmoid(2*t_e - m2),
    t = l - max(l).  Gates scale mid in-place (per-expert per-partition
    scalar multiply) with fp16 output; the LoRA scale 4.0 is folded into B.
  - mid.T is restored with a tiny fp16 PE transpose, then mm2 (fp16) computes
    delta[t, d] = midT.T @ (4*B_all); results are written out as fp16 and
    upcast to fp32 on the host (max elementwise error ~1e-3 << 2e-2 gate).
"""

import os
import sys

for _p in ("/opt/trn_rl_repo", "/root/.axon_site/_ro/trn_rl_repo"):
    if os.path.isdir(_p) and _p not in sys.path:
        sys.path.insert(0, _p)

import numpy as np
from contextlib import ExitStack

import concourse.bass as bass
import concourse.bacc as bacc
import concourse.mybir as mybir
import concourse.tile as tile

N_CORES = 8
B_, S, D = 4, 2048, 3840
T_FULL = B_ * S                 # 8192
T_C = T_FULL // N_CORES         # 1024 tokens per core
E, R = 4, 16
ER = E * R                      # 64
M_W = ER + E                    # 68 = A rows + router rows
LORA_SCALE = 16.0 / np.sqrt(16.0)   # 4.0 (folded into B on the host)

GROUP = 256                     # tokens per group
TPG = GROUP // 128              # token tiles per group (2)
N_GROUPS = T_C // GROUP         # 4
D_CHUNKS = D // 128             # 30
TP_C = 3                        # chunks per transpose-psum tile
MM2_CHUNKS = [(i * 512, min(512, D - i * 512)) for i in range((D + 511) // 512)]

F32 = mybir.dt.float32
F16 = mybir.dt.float16
BF16 = mybir.dt.bfloat16


def build_kernel(tc: tile.TileContext, out_d, x_d, wta_d, rw_d, b_d, id_d,
                 id16_d):
    nc = tc.nc
    with ExitStack() as ctx:
        const_pool = ctx.enter_context(tc.tile_pool(name="const", bufs=1))
        xin_pool = ctx.enter_context(tc.tile_pool(name="xin", bufs=3))
        xt_pool = ctx.enter_context(tc.tile_pool(name="xt", bufs=2))
        xt16_pool = ctx.enter_context(tc.tile_pool(name="xt16", bufs=2))
        mts_pool = ctx.enter_context(tc.tile_pool(name="mts", bufs=2))
        g_pool = ctx.enter_context(tc.tile_pool(name="gate", bufs=2))
        dout_pool = ctx.enter_context(tc.tile_pool(name="dout", bufs=2))
        ps_tp = ctx.enter_context(
            tc.tile_pool(name="ps_tp", bufs=3, space=bass.MemorySpace.PSUM))
        ps_mid = ctx.enter_context(
            tc.tile_pool(name="ps_mid", bufs=2, space=bass.MemorySpace.PSUM))
        ps_mt = ctx.enter_context(
            tc.tile_pool(name="ps_mt", bufs=1, space=bass.MemorySpace.PSUM))
        ps_mm2 = ctx.enter_context(
            tc.tile_pool(name="ps_mm2", bufs=2, space=bass.MemorySpace.PSUM))

        # ---- constants / weights ----
        # identities first (needed by the first transposes); the big weight
        # loads are emitted after group 0's x tiles (emit_weights below) so
        # they stay off the startup critical path.
        id_sb = const_pool.tile([128, 128], F32, tag="ident")
        nc.sync.dma_start(id_sb[:], id_d[:])
        id16_sb = const_pool.tile([128, 128], F16, tag="ident16")
        nc.sync.dma_start(id16_sb[:], id16_d[:])
        wta_sb = const_pool.tile([128, D_CHUNKS, ER], BF16, tag="wta")
        rw_sb = const_pool.tile([128, D_CHUNKS, E], F32, tag="rw")
        b_sb = const_pool.tile([ER, D], F16, tag="ball")

        def emit_weights():
            nc.sync.dma_start(rw_sb[:], rw_d[:])
            nc.sync.dma_start(wta_sb[:], wta_d[:])
            nc.sync.dma_start(b_sb[:], b_d[:])

        copy_engines = [nc.vector, nc.scalar]
        cp_state = {"i": 0}

        def copy_any(dst, src):
            eng = copy_engines[cp_state["i"] % 2]
            cp_state["i"] += 1
            if eng is nc.vector:
                eng.tensor_copy(dst, src)
            else:
                eng.copy(dst, src)

        HC = D_CHUNKS // 2          # chunks per x half-tile (15)

        def emit_loads(g):
            """DMA x half-tiles, PE-transpose, copy to SBUF xt (fp32); a
            Pool-engine bulk copy then rounds each half to bf16 for mm1."""
            tok_g = g * GROUP
            xt_sb = xt_pool.tile([128, D_CHUNKS, GROUP], F32, tag="xt")
            xt16_sb = xt16_pool.tile([128, D_CHUNKS, GROUP], BF16, tag="xt16")
            for h in range(2):
                for tl in range(TPG):
                    tok0 = tok_g + tl * 128
                    x_sb = xin_pool.tile([128, HC * 128], F32, tag="xin")
                    nc.sync.dma_start(
                        x_sb[:],
                        x_d[tok0:tok0 + 128, h * HC * 128:(h + 1) * HC * 128])
                    for c0 in range(0, HC, TP_C):
                        tp_ps = ps_tp.tile([128, TP_C, 128], F32, tag="tp")
                        for cc in range(TP_C):
                            c = c0 + cc
                            nc.tensor.transpose(
                                tp_ps[:, cc, :],
                                x_sb[:, c * 128:(c + 1) * 128], id_sb[:])
                        copy_any(
                            xt_sb[:, h * HC + c0:h * HC + c0 + TP_C,
                                  tl * 128:(tl + 1) * 128],
                            tp_ps[:])
                nc.gpsimd.tensor_copy(
                    xt16_sb[:, h * HC:(h + 1) * HC, :],
                    xt_sb[:, h * HC:(h + 1) * HC, :])
            return xt_sb, xt16_sb

        def emit_mm1(g, xt_sb, xt16_sb):
            """Token-major matmuls: exact fp32 router logits (4 cols) plus
            bf16 mid = x_tile @ A.T (64 cols), sharing one PSUM tile."""
            mid_ps = ps_mid.tile([128, TPG, M_W], F32, tag="mid")
            for tl in range(TPG):
                for c in range(D_CHUNKS):
                    nc.tensor.matmul(
                        mid_ps[:, tl, ER:M_W],
                        xt_sb[:, c, tl * 128:(tl + 1) * 128],
                        rw_sb[:, c, :],
                        start=(c == 0),
                        stop=(c == D_CHUNKS - 1),
                    )
            for tl in range(TPG):
                for c in range(D_CHUNKS):
                    nc.tensor.matmul(
                        mid_ps[:, tl, 0:ER],
                        xt16_sb[:, c, tl * 128:(tl + 1) * 128],
                        wta_sb[:, c, :],
                        start=(c == 0),
                        stop=(c == D_CHUNKS - 1),
                    )
            return mid_ps

        def emit_gating(g, mid_ps):
            """Top-2 softmax gating, token-major, and gate-scaled fp16 mid."""
            mts_tm = []
            for tl in range(TPG):
                L = g_pool.tile([128, E], F32, tag="L")
                nc.vector.tensor_copy(L[:], mid_ps[:, tl, ER:M_W])
                m1 = g_pool.tile([128, 1], F32, tag="m1")
                nc.vector.tensor_reduce(
                    m1[:], L[:], axis=mybir.AxisListType.X,
                    op=mybir.AluOpType.max)
                tt = g_pool.tile([128, E], F32, tag="tt")
                nc.vector.tensor_scalar(
                    tt[:], L[:], m1[:], None, op0=mybir.AluOpType.subtract)
                z = g_pool.tile([128, E], F32, tag="z")
                nc.vector.tensor_scalar(
                    z[:], tt[:], 0.0, None, op0=mybir.AluOpType.is_equal)
                msk = g_pool.tile([128, E], F32, tag="msk")
                nc.vector.scalar_tensor_tensor(
                    msk[:], z[:], -1e30, tt[:],
                    op0=mybir.AluOpType.mult, op1=mybir.AluOpType.add)
                m2 = g_pool.tile([128, 1], F32, tag="m2")
                nc.vector.tensor_reduce(
                    m2[:], msk[:], axis=mybir.AxisListType.X,
                    op=mybir.AluOpType.max)
                s2 = g_pool.tile([128, E], F32, tag="s2")
                nc.vector.tensor_scalar(
                    s2[:], tt[:], 2.0, m2[:],
                    op0=mybir.AluOpType.mult, op1=mybir.AluOpType.subtract)
                sg = g_pool.tile([128, E], F32, tag="sg")
                nc.scalar.activation(
                    sg[:], s2[:], mybir.ActivationFunctionType.Sigmoid)
                ge = g_pool.tile([128, E], F32, tag="ge")
                nc.vector.tensor_scalar(
                    ge[:], tt[:], m2[:], None, op0=mybir.AluOpType.is_ge)
                gates = g_pool.tile([128, E], F32, tag="gates")
                nc.vector.tensor_tensor(
                    gates[:], ge[:], sg[:], op=mybir.AluOpType.mult)

                # scale mid rows by the per-expert gate (per-partition scalar)
                mt = mts_pool.tile([128, ER], F16, tag="mtm")
                for e in range(E):
                    nc.vector.tensor_scalar(
                        mt[:, e * R:(e + 1) * R],
                        mid_ps[:, tl, e * R:(e + 1) * R],
                        gates[:, e:e + 1], None,
                        op0=mybir.AluOpType.mult)
                mts_tm.append(mt)
            return mts_tm

        def emit_tail(g, mts_tm):
            """Transpose gated mid back to [er, t] (fp16), mm2, write out."""
            tok_g = g * GROUP
            mt_ps = ps_mt.tile([ER, TPG, 128], F16, tag="mt")
            mts_sb = mts_pool.tile([ER, TPG, 128], F16, tag="mts")
            for tl in range(TPG):
                nc.tensor.transpose(
                    mt_ps[:, tl, :], mts_tm[tl][:], id16_sb[:])
                copy_any(mts_sb[:, tl, :], mt_ps[:, tl, :])
            for tl in range(TPG):
                tok0 = tok_g + tl * 128
                dout_sb = dout_pool.tile([128, D], F16, tag="dout")
                for (d0, w) in MM2_CHUNKS:
                    mm2_ps = ps_mm2.tile([128, 512], F32, tag="mm2")
                    nc.tensor.matmul(
                        mm2_ps[:, 0:w],
                        mts_sb[:, tl, :],
                        b_sb[:, d0:d0 + w],
                    )
                    copy_any(dout_sb[:, d0:d0 + w], mm2_ps[:, 0:w])
                # stores go out via the (otherwise idle) Pool engine's SWDGE
                # path so they never head-of-line-block the SP load queue
                nc.gpsimd.dma_start(out_d[tok0:tok0 + 128, :], dout_sb[:])

        # Software pipeline: loads for group g+1 are emitted between group
        # g's gating (DVE) and its mm2, so the PE transposes of g+1 overlap
        # the gating chain and DMA stays saturated.
        xt = emit_loads(0)
        emit_weights()
        for g in range(N_GROUPS):
            mid_ps = emit_mm1(g, *xt)
            mts_tm = emit_gating(g, mid_ps)
            if g + 1 < N_GROUPS:
                xt = emit_loads(g + 1)
            emit_tail(g, mts_tm)


_CACHED = {}


def _build_module():
    key = "tm"
    if key in _CACHED:
        return _CACHED[key]
    nc = bacc.Bacc("TRN2", target_bir_lowering=False, debug=False)
    x_d = nc.dram_tensor("x_in", [T_C, D], F32, kind="ExternalInput").ap()
    wta_d = nc.dram_tensor(
        "wta_in", [128, D_CHUNKS, ER], BF16, kind="ExternalInput").ap()
    rw_d = nc.dram_tensor(
        "rw_in", [128, D_CHUNKS, E], F32, kind="ExternalInput").ap()
    b_d = nc.dram_tensor("ball_in", [ER, D], F16, kind="ExternalInput").ap()
    id_d = nc.dram_tensor("id_in", [128, 128], F32, kind="ExternalInput").ap()
    id16_d = nc.dram_tensor(
        "id16_in", [128, 128], F16, kind="ExternalInput").ap()
    out_d = nc.dram_tensor("out", [T_C, D], F16, kind="ExternalOutput").ap()
    with tile.TileContext(nc) as tc:
        build_kernel(tc, out_d, x_d, wta_d, rw_d, b_d, id_d, id16_d)
    nc.compile()
    _CACHED[key] = nc
    return nc


def _host_weights(router_w, A, B):
    import ml_dtypes
    # [128, D_CHUNKS, M]: element [p, c, m] = W.T[c*128 + p, m]
    WA = np.ascontiguousarray(
        A.reshape(ER, D).T.reshape(D_CHUNKS, 128, ER).transpose(1, 0, 2)
    ).astype(ml_dtypes.bfloat16)
    RW = np.ascontiguousarray(
        router_w.T.reshape(D_CHUNKS, 128, E).transpose(1, 0, 2)
    ).astype(np.float32)
    B_all = np.ascontiguousarray(
        (B.transpose(0, 2, 1).reshape(ER, D) * LORA_SCALE)).astype(np.float16)
    ident = np.eye(128, dtype=np.float32)
    ident16 = np.eye(128, dtype=np.float16)
    return WA, RW, B_all, ident, ident16


def make_in_maps(x, router_w, A, B):
    flat = np.ascontiguousarray(np.asarray(x, np.float32).reshape(T_FULL, D))
    WA, RW, B_all, ident, ident16 = _host_weights(
        np.asarray(router_w, np.float32),
        np.asarray(A, np.float32),
        np.asarray(B, np.float32))
    in_maps = []
    for i in range(N_CORES):
        in_maps.append({
            "x_in": flat[i * T_C:(i + 1) * T_C],
            "wta_in": WA,
            "rw_in": RW,
            "ball_in": B_all,
            "id_in": ident,
            "id16_in": ident16,
        })
    return in_maps


def kernel(x, router_w, A, B, _results_hook=None):
    from concourse.bass_utils import run_bass_kernel_spmd

    nc = _build_module()
    in_maps = make_in_maps(x, router_w, A, B)
    res = run_bass_kernel_spmd(nc, in_maps, core_ids=list(range(N_CORES)))
    if _results_hook is not None:
        _results_hook(res)
    out = np.concatenate(
        [np.asarray(res.results[i]["out"]) for i in range(N_CORES)], axis=0)
    return out.astype(np.float32).reshape(B_, S, D)


if __name__ == "__main__":
    rng = np.random.default_rng(0)
    x = rng.standard_normal((B_, S, D), dtype=np.float32)
    rw = (rng.standard_normal((E, D)) * 0.02).astype(np.float32)
    A = (rng.standard_normal((E, R, D)) * 0.02).astype(np.float32)
    Bm = (rng.standard_normal((E, D, R)) * 0.02).astype(np.float32)
    out = kernel(x, rw, A, Bm)
    print("out", out.shape, out.dtype, float(np.abs(out).max()))


# revision 17
# speedup vs baseline: 1.7517x; 1.0026x over previous
"""MoE LoRA delta kernel for Trainium2 (8 NeuronCores, data-parallel over tokens).

Computation (per token t):
    logits = x @ router_w.T                      [T, 4]
    gates  = top2-softmax(logits)                [T, 4]  (exactly 2 nonzero)
    mid    = x @ A_all.T                         [T, 64]   A_all[(e,r), d]
    delta  = (mid * expand(gates)) @ (4*B_all)   [T, D]    B_all[(e,r), d]

Kernel strategy per core (T_c = 1024 tokens, groups of 256):
  - x tiles are transposed on-chip (PE transpose, fp32 exact) into xt [D, T].
  - One fused fp32 matmul chain per 128-token tile computes token-major
    [128, 68] = x_tile @ [A_all.T | router_w.T]: cols 0:64 are mid, cols
    64:68 are the router logits (fp32-exact so the top-2 expert selection
    matches the fp32 reference).
  - Gating runs token-major in fp32: g_e = 1{t_e >= m2} * sigmoid(2*t_e - m2),
    t = l - max(l).  Gates scale mid in-place (per-expert per-partition
    scalar multiply) with fp16 output; the LoRA scale 4.0 is folded into B.
  - mid.T is restored with a tiny fp16 PE transpose, then mm2 (fp16) computes
    delta[t, d] = midT.T @ (4*B_all); results are written out as fp16 and
    upcast to fp32 on the host (max elementwise error ~1e-3 << 2e-2 gate).
"""

import os
import sys

for _p in ("/opt/trn_rl_repo", "/root/.axon_site/_ro/trn_rl_repo"):
    if os.path.isdir(_p) and _p not in sys.path:
        sys.path.insert(0, _p)

import numpy as np
from contextlib import ExitStack

import concourse.bass as bass
import concourse.bacc as bacc
import concourse.mybir as mybir
import concourse.tile as tile

N_CORES = 8
B_, S, D = 4, 2048, 3840
T_FULL = B_ * S                 # 8192
T_C = T_FULL // N_CORES         # 1024 tokens per core
E, R = 4, 16
ER = E * R                      # 64
M_W = ER + E                    # 68 = A rows + router rows
LORA_SCALE = 16.0 / np.sqrt(16.0)   # 4.0 (folded into B on the host)

GROUP = 256                     # tokens per group
TPG = GROUP // 128              # token tiles per group (2)
N_GROUPS = T_C // GROUP         # 4
D_CHUNKS = D // 128             # 30
TP_C = 3                        # chunks per transpose-psum tile
MM2_CHUNKS = [(i * 512, min(512, D - i * 512)) for i in range((D + 511) // 512)]

F32 = mybir.dt.float32
F16 = mybir.dt.float16
BF16 = mybir.dt.bfloat16


def build_kernel(tc: tile.TileContext, out_d, x_d, wta_d, rw_d, b_d, id_d,
                 id16_d):
    nc = tc.nc
    with ExitStack() as ctx:
        const_pool = ctx.enter_context(tc.tile_pool(name="const", bufs=1))
        xin_pool = ctx.enter_context(tc.tile_pool(name="xin", bufs=3))
        xt_pool = ctx.enter_context(tc.tile_pool(name="xt", bufs=3))
        xt16_pool = ctx.enter_context(tc.tile_pool(name="xt16", bufs=3))
        mts_pool = ctx.enter_context(tc.tile_pool(name="mts", bufs=2))
        g_pool = ctx.enter_context(tc.tile_pool(name="gate", bufs=2))
        dout_pool = ctx.enter_context(tc.tile_pool(name="dout", bufs=2))
        ps_tp = ctx.enter_context(
            tc.tile_pool(name="ps_tp", bufs=3, space=bass.MemorySpace.PSUM))
        ps_mid = ctx.enter_context(
            tc.tile_pool(name="ps_mid", bufs=2, space=bass.MemorySpace.PSUM))
        ps_mt = ctx.enter_context(
            tc.tile_pool(name="ps_mt", bufs=1, space=bass.MemorySpace.PSUM))
        ps_mm2 = ctx.enter_context(
            tc.tile_pool(name="ps_mm2", bufs=2, space=bass.MemorySpace.PSUM))

        # ---- constants / weights ----
        # identities first (needed by the first transposes); the big weight
        # loads are emitted after group 0's x tiles (emit_weights below) so
        # they stay off the startup critical path.
        id_sb = const_pool.tile([128, 128], F32, tag="ident")
        nc.sync.dma_start(id_sb[:], id_d[:])
        id16_sb = const_pool.tile([128, 128], F16, tag="ident16")
        nc.sync.dma_start(id16_sb[:], id16_d[:])
        wta_sb = const_pool.tile([128, D_CHUNKS, ER], BF16, tag="wta")
        rw_sb = const_pool.tile([128, D_CHUNKS, E], F32, tag="rw")
        b_sb = const_pool.tile([ER, D], F16, tag="ball")

        def emit_weights():
            nc.sync.dma_start(rw_sb[:], rw_d[:])
            nc.sync.dma_start(wta_sb[:], wta_d[:])
            nc.sync.dma_start(b_sb[:], b_d[:])

        copy_engines = [nc.vector, nc.scalar]
        cp_state = {"i": 0}

        def copy_any(dst, src):
            eng = copy_engines[cp_state["i"] % 2]
            cp_state["i"] += 1
            if eng is nc.vector:
                eng.tensor_copy(dst, src)
            else:
                eng.copy(dst, src)

        HC = D_CHUNKS // 2          # chunks per x half-tile (15)

        def emit_loads(g):
            """DMA x half-tiles, PE-transpose, copy to SBUF xt (fp32); a
            Pool-engine bulk copy then rounds each half to bf16 for mm1."""
            tok_g = g * GROUP
            xt_sb = xt_pool.tile([128, D_CHUNKS, GROUP], F32, tag="xt")
            xt16_sb = xt16_pool.tile([128, D_CHUNKS, GROUP], BF16, tag="xt16")
            for h in range(2):
                for tl in range(TPG):
                    tok0 = tok_g + tl * 128
                    x_sb = xin_pool.tile([128, HC * 128], F32, tag="xin")
                    nc.sync.dma_start(
                        x_sb[:],
                        x_d[tok0:tok0 + 128, h * HC * 128:(h + 1) * HC * 128])
                    for c0 in range(0, HC, TP_C):
                        tp_ps = ps_tp.tile([128, TP_C, 128], F32, tag="tp")
                        for cc in range(TP_C):
                            c = c0 + cc
                            nc.tensor.transpose(
                                tp_ps[:, cc, :],
                                x_sb[:, c * 128:(c + 1) * 128], id_sb[:])
                        copy_any(
                            xt_sb[:, h * HC + c0:h * HC + c0 + TP_C,
                                  tl * 128:(tl + 1) * 128],
                            tp_ps[:])
                nc.gpsimd.tensor_copy(
                    xt16_sb[:, h * HC:(h + 1) * HC, :],
                    xt_sb[:, h * HC:(h + 1) * HC, :])
            return xt_sb, xt16_sb

        def emit_mm1(g, xt_sb, xt16_sb):
            """Token-major matmuls: exact fp32 router logits (4 cols) plus
            bf16 mid = x_tile @ A.T (64 cols), sharing one PSUM tile."""
            mid_ps = ps_mid.tile([128, TPG, M_W], F32, tag="mid")
            for tl in range(TPG):
                for c in range(D_CHUNKS):
                    nc.tensor.matmul(
                        mid_ps[:, tl, ER:M_W],
                        xt_sb[:, c, tl * 128:(tl + 1) * 128],
                        rw_sb[:, c, :],
                        start=(c == 0),
                        stop=(c == D_CHUNKS - 1),
                    )
            for tl in range(TPG):
                for c in range(D_CHUNKS):
                    nc.tensor.matmul(
                        mid_ps[:, tl, 0:ER],
                        xt16_sb[:, c, tl * 128:(tl + 1) * 128],
                        wta_sb[:, c, :],
                        start=(c == 0),
                        stop=(c == D_CHUNKS - 1),
                    )
            return mid_ps

        def emit_gating(g, mid_ps):
            """Top-2 softmax gating, token-major, and gate-scaled fp16 mid."""
            mts_tm = []
            for tl in range(TPG):
                L = g_pool.tile([128, E], F32, tag="L")
                nc.vector.tensor_copy(L[:], mid_ps[:, tl, ER:M_W])
                m1 = g_pool.tile([128, 1], F32, tag="m1")
                nc.vector.tensor_reduce(
                    m1[:], L[:], axis=mybir.AxisListType.X,
                    op=mybir.AluOpType.max)
                tt = g_pool.tile([128, E], F32, tag="tt")
                nc.vector.tensor_scalar(
                    tt[:], L[:], m1[:], None, op0=mybir.AluOpType.subtract)
                z = g_pool.tile([128, E], F32, tag="z")
                nc.vector.tensor_scalar(
                    z[:], tt[:], 0.0, None, op0=mybir.AluOpType.is_equal)
                msk = g_pool.tile([128, E], F32, tag="msk")
                nc.vector.scalar_tensor_tensor(
                    msk[:], z[:], -1e30, tt[:],
                    op0=mybir.AluOpType.mult, op1=mybir.AluOpType.add)
                m2 = g_pool.tile([128, 1], F32, tag="m2")
                nc.vector.tensor_reduce(
                    m2[:], msk[:], axis=mybir.AxisListType.X,
                    op=mybir.AluOpType.max)
                s2 = g_pool.tile([128, E], F32, tag="s2")
                nc.vector.tensor_scalar(
                    s2[:], tt[:], 2.0, m2[:],
                    op0=mybir.AluOpType.mult, op1=mybir.AluOpType.subtract)
                sg = g_pool.tile([128, E], F32, tag="sg")
                nc.scalar.activation(
                    sg[:], s2[:], mybir.ActivationFunctionType.Sigmoid)
                ge = g_pool.tile([128, E], F32, tag="ge")
                nc.vector.tensor_scalar(
                    ge[:], tt[:], m2[:], None, op0=mybir.AluOpType.is_ge)
                gates = g_pool.tile([128, E], F32, tag="gates")
                nc.vector.tensor_tensor(
                    gates[:], ge[:], sg[:], op=mybir.AluOpType.mult)

                # scale mid rows by the per-expert gate (per-partition scalar)
                mt = mts_pool.tile([128, ER], F16, tag="mtm")
                for e in range(E):
                    nc.vector.tensor_scalar(
                        mt[:, e * R:(e + 1) * R],
                        mid_ps[:, tl, e * R:(e + 1) * R],
                        gates[:, e:e + 1], None,
                        op0=mybir.AluOpType.mult)
                mts_tm.append(mt)
            return mts_tm

        def emit_tail(g, mts_tm):
            """Transpose gated mid back to [er, t] (fp16), mm2, write out."""
            tok_g = g * GROUP
            mt_ps = ps_mt.tile([ER, TPG, 128], F16, tag="mt")
            mts_sb = mts_pool.tile([ER, TPG, 128], F16, tag="mts")
            for tl in range(TPG):
                nc.tensor.transpose(
                    mt_ps[:, tl, :], mts_tm[tl][:], id16_sb[:])
                copy_any(mts_sb[:, tl, :], mt_ps[:, tl, :])
            for tl in range(TPG):
                tok0 = tok_g + tl * 128
                dout_sb = dout_pool.tile([128, D], F16, tag="dout")
                for (d0, w) in MM2_CHUNKS:
                    mm2_ps = ps_mm2.tile([128, 512], F32, tag="mm2")
                    nc.tensor.matmul(
                        mm2_ps[:, 0:w],
                        mts_sb[:, tl, :],
                        b_sb[:, d0:d0 + w],
                    )
                    copy_any(dout_sb[:, d0:d0 + w], mm2_ps[:, 0:w])
                # stores go out via the (otherwise idle) Pool engine's SWDGE
                # path so they never head-of-line-block the SP load queue
                nc.gpsimd.dma_start(out_d[tok0:tok0 + 128, :], dout_sb[:])

        # Two-group software pipeline: loads (DMA + PE transposes + copies +
        # Pool bf16 conversion) run two groups ahead of compute, hiding the
        # conversion latency; DMA stays saturated throughout.
        xt_a = emit_loads(0)
        emit_weights()
        xt_b = emit_loads(1)
        for g in range(N_GROUPS):
            mid_ps = emit_mm1(g, *xt_a)
            mts_tm = emit_gating(g, mid_ps)
            if g + 2 < N_GROUPS:
                xt_a, xt_b = xt_b, emit_loads(g + 2)
            else:
                xt_a, xt_b = xt_b, None
            emit_tail(g, mts_tm)


_CACHED = {}


def _build_module():
    key = "tm"
    if key in _CACHED:
        return _CACHED[key]
    nc = bacc.Bacc("TRN2", target_bir_lowering=False, debug=False)
    x_d = nc.dram_tensor("x_in", [T_C, D], F32, kind="ExternalInput").ap()
    wta_d = nc.dram_tensor(
        "wta_in", [128, D_CHUNKS, ER], BF16, kind="ExternalInput").ap()
    rw_d = nc.dram_tensor(
        "rw_in", [128, D_CHUNKS, E], F32, kind="ExternalInput").ap()
    b_d = nc.dram_tensor("ball_in", [ER, D], F16, kind="ExternalInput").ap()
    id_d = nc.dram_tensor("id_in", [128, 128], F32, kind="ExternalInput").ap()
    id16_d = nc.dram_tensor(
        "id16_in", [128, 128], F16, kind="ExternalInput").ap()
    out_d = nc.dram_tensor("out", [T_C, D], F16, kind="ExternalOutput").ap()
    with tile.TileContext(nc) as tc:
        build_kernel(tc, out_d, x_d, wta_d, rw_d, b_d, id_d, id16_d)
    nc.compile()
    _CACHED[key] = nc
    return nc


def _host_weights(router_w, A, B):
    import ml_dtypes
    # [128, D_CHUNKS, M]: element [p, c, m] = W.T[c*128 + p, m]
    WA = np.ascontiguousarray(
        A.reshape(ER, D).T.reshape(D_CHUNKS, 128, ER).transpose(1, 0, 2)
    ).astype(ml_dtypes.bfloat16)
    RW = np.ascontiguousarray(
        router_w.T.reshape(D_CHUNKS, 128, E).transpose(1, 0, 2)
    ).astype(np.float32)
    B_all = np.ascontiguousarray(
        (B.transpose(0, 2, 1).reshape(ER, D) * LORA_SCALE)).astype(np.float16)
    ident = np.eye(128, dtype=np.float32)
    ident16 = np.eye(128, dtype=np.float16)
    return WA, RW, B_all, ident, ident16


def make_in_maps(x, router_w, A, B):
    flat = np.ascontiguousarray(np.asarray(x, np.float32).reshape(T_FULL, D))
    WA, RW, B_all, ident, ident16 = _host_weights(
        np.asarray(router_w, np.float32),
        np.asarray(A, np.float32),
        np.asarray(B, np.float32))
    in_maps = []
    for i in range(N_CORES):
        in_maps.append({
            "x_in": flat[i * T_C:(i + 1) * T_C],
            "wta_in": WA,
            "rw_in": RW,
            "ball_in": B_all,
            "id_in": ident,
            "id16_in": ident16,
        })
    return in_maps


def kernel(x, router_w, A, B, _results_hook=None):
    from concourse.bass_utils import run_bass_kernel_spmd

    nc = _build_module()
    in_maps = make_in_maps(x, router_w, A, B)
    res = run_bass_kernel_spmd(nc, in_maps, core_ids=list(range(N_CORES)))
    if _results_hook is not None:
        _results_hook(res)
    out = np.concatenate(
        [np.asarray(res.results[i]["out"]) for i in range(N_CORES)], axis=0)
    return out.astype(np.float32).reshape(B_, S, D)


if __name__ == "__main__":
    rng = np.random.default_rng(0)
    x = rng.standard_normal((B_, S, D), dtype=np.float32)
    rw = (rng.standard_normal((E, D)) * 0.02).astype(np.float32)
    A = (rng.standard_normal((E, R, D)) * 0.02).astype(np.float32)
    Bm = (rng.standard_normal((E, D, R)) * 0.02).astype(np.float32)
    out = kernel(x, rw, A, Bm)
    print("out", out.shape, out.dtype, float(np.abs(out).max()))


# revision 21
# speedup vs baseline: 1.7887x; 1.0211x over previous
"""MoE LoRA delta kernel for Trainium2 (8 NeuronCores, data-parallel over tokens).

Computation (per token t):
    logits = x @ router_w.T                      [T, 4]
    gates  = top2-softmax(logits)                [T, 4]  (exactly 2 nonzero)
    mid    = x @ A_all.T                         [T, 64]   A_all[(e,r), d]
    delta  = (mid * expand(gates)) @ (4*B_all)   [T, D]    B_all[(e,r), d]

Kernel strategy per core (T_c = 1024 tokens, groups of 256):
  - x tiles are transposed on-chip (PE transpose, fp32 exact) into xt [D, T].
  - One fused fp32 matmul chain per 128-token tile computes token-major
    [128, 68] = x_tile @ [A_all.T | router_w.T]: cols 0:64 are mid, cols
    64:68 are the router logits (fp32-exact so the top-2 expert selection
    matches the fp32 reference).
  - Gating runs token-major in fp32: g_e = 1{t_e >= m2} * sigmoid(2*t_e - m2),
    t = l - max(l).  Gates scale mid in-place (per-expert per-partition
    scalar multiply) with fp16 output; the LoRA scale 4.0 is folded into B.
  - mid.T is restored with a tiny fp16 PE transpose, then mm2 (fp16) computes
    delta[t, d] = midT.T @ (4*B_all); results are written out as fp16 and
    upcast to fp32 on the host (max elementwise error ~1e-3 << 2e-2 gate).
"""

import os
import sys

for _p in ("/opt/trn_rl_repo", "/root/.axon_site/_ro/trn_rl_repo"):
    if os.path.isdir(_p) and _p not in sys.path:
        sys.path.insert(0, _p)

import numpy as np
from contextlib import ExitStack

import concourse.bass as bass
import concourse.bacc as bacc
import concourse.mybir as mybir
import concourse.tile as tile

N_CORES = 8
B_, S, D = 4, 2048, 3840
T_FULL = B_ * S                 # 8192
T_C = T_FULL // N_CORES         # 1024 tokens per core
E, R = 4, 16
ER = E * R                      # 64
M_W = ER + E                    # 68 = A rows + router rows
LORA_SCALE = 16.0 / np.sqrt(16.0)   # 4.0 (folded into B on the host)

GROUP = 256                     # tokens per group
TPG = GROUP // 128              # token tiles per group (2)
N_GROUPS = T_C // GROUP         # 4
D_CHUNKS = D // 128             # 30
TP_C = 3                        # chunks per transpose-psum tile
MM2_CHUNKS = [(i * 512, min(512, D - i * 512)) for i in range((D + 511) // 512)]

F32 = mybir.dt.float32
F16 = mybir.dt.float16
BF16 = mybir.dt.bfloat16


def build_kernel(tc: tile.TileContext, out_d, x_d, wta_d, rw_d, b_d, id_d,
                 id16_d):
    nc = tc.nc
    with ExitStack() as ctx:
        const_pool = ctx.enter_context(tc.tile_pool(name="const", bufs=1))
        xin_pool = ctx.enter_context(tc.tile_pool(name="xin", bufs=6))
        xt_pool = ctx.enter_context(tc.tile_pool(name="xt", bufs=3))
        xt16_pool = ctx.enter_context(tc.tile_pool(name="xt16", bufs=2))
        mts_pool = ctx.enter_context(tc.tile_pool(name="mts", bufs=2))
        g_pool = ctx.enter_context(tc.tile_pool(name="gate", bufs=2))
        dout_pool = ctx.enter_context(tc.tile_pool(name="dout", bufs=2))
        ps_tp = ctx.enter_context(
            tc.tile_pool(name="ps_tp", bufs=3, space=bass.MemorySpace.PSUM))
        ps_mid = ctx.enter_context(
            tc.tile_pool(name="ps_mid", bufs=2, space=bass.MemorySpace.PSUM))
        ps_mt = ctx.enter_context(
            tc.tile_pool(name="ps_mt", bufs=1, space=bass.MemorySpace.PSUM))
        ps_mm2 = ctx.enter_context(
            tc.tile_pool(name="ps_mm2", bufs=2, space=bass.MemorySpace.PSUM))

        # ---- constants / weights ----
        # identities first (needed by the first transposes); the big weight
        # loads are emitted after group 0's x tiles (emit_weights below) so
        # they stay off the startup critical path.
        id_sb = const_pool.tile([128, 128], F32, tag="ident")
        nc.sync.dma_start(id_sb[:], id_d[:])
        id16_sb = const_pool.tile([128, 128], F16, tag="ident16")
        nc.sync.dma_start(id16_sb[:], id16_d[:])
        wta_sb = const_pool.tile([128, D_CHUNKS, ER], BF16, tag="wta")
        rw_sb = const_pool.tile([128, D_CHUNKS, E], F32, tag="rw")
        b_sb = const_pool.tile([ER, D], F16, tag="ball")

        def emit_weights():
            nc.sync.dma_start(rw_sb[:], rw_d[:])
            nc.sync.dma_start(wta_sb[:], wta_d[:])
            nc.sync.dma_start(b_sb[:], b_d[:])

        copy_engines = [nc.vector, nc.scalar]
        cp_state = {"i": 0}

        def copy_any(dst, src):
            eng = copy_engines[cp_state["i"] % 2]
            cp_state["i"] += 1
            if eng is nc.vector:
                eng.tensor_copy(dst, src)
            else:
                eng.copy(dst, src)

        HC = D_CHUNKS // 2          # chunks per x half-tile (15)

        def emit_loads(g):
            """DMA x half-tiles, PE-transpose, copy to SBUF xt (fp32); a
            Pool-engine bulk copy then rounds each half to bf16 for mm1."""
            tok_g = g * GROUP
            xt_sb = xt_pool.tile([128, D_CHUNKS, GROUP], F32, tag="xt")
            xt16_sb = xt16_pool.tile([128, D_CHUNKS, GROUP], BF16, tag="xt16")
            for h in range(2):
                for tl in range(TPG):
                    tok0 = tok_g + tl * 128
                    x_sb = xin_pool.tile([128, HC * 128], F32, tag="xin")
                    nc.sync.dma_start(
                        x_sb[:],
                        x_d[tok0:tok0 + 128, h * HC * 128:(h + 1) * HC * 128])
                    for c0 in range(0, HC, TP_C):
                        tp_ps = ps_tp.tile([128, TP_C, 128], F32, tag="tp")
                        for cc in range(TP_C):
                            c = c0 + cc
                            nc.tensor.transpose(
                                tp_ps[:, cc, :],
                                x_sb[:, c * 128:(c + 1) * 128], id_sb[:])
                        copy_any(
                            xt_sb[:, h * HC + c0:h * HC + c0 + TP_C,
                                  tl * 128:(tl + 1) * 128],
                            tp_ps[:])
                nc.gpsimd.tensor_copy(
                    xt16_sb[:, h * HC:(h + 1) * HC, :],
                    xt_sb[:, h * HC:(h + 1) * HC, :])
            return xt_sb, xt16_sb

        def emit_logits(g, xt_sb):
            """Exact fp32 router logits, token-major ([128, 4] per ttile)."""
            mid_ps = ps_mid.tile([128, TPG, M_W], F32, tag="mid")
            for tl in range(TPG):
                for c in range(D_CHUNKS):
                    nc.tensor.matmul(
                        mid_ps[:, tl, ER:M_W],
                        xt_sb[:, c, tl * 128:(tl + 1) * 128],
                        rw_sb[:, c, :],
                        start=(c == 0),
                        stop=(c == D_CHUNKS - 1),
                    )
            return mid_ps

        def emit_mid(g, mid_ps, xt16_sb):
            """bf16 mid = x_tile @ A.T (64 cols, token-major)."""
            for tl in range(TPG):
                for c in range(D_CHUNKS):
                    nc.tensor.matmul(
                        mid_ps[:, tl, 0:ER],
                        xt16_sb[:, c, tl * 128:(tl + 1) * 128],
                        wta_sb[:, c, :],
                        start=(c == 0),
                        stop=(c == D_CHUNKS - 1),
                    )

        def emit_gates(g, mid_ps):
            """Top-2 softmax gating, token-major."""
            gates_tl = []
            for tl in range(TPG):
                L = g_pool.tile([128, E], F32, tag="L")
                nc.vector.tensor_copy(L[:], mid_ps[:, tl, ER:M_W])
                m1 = g_pool.tile([128, 1], F32, tag="m1")
                nc.vector.tensor_reduce(
                    m1[:], L[:], axis=mybir.AxisListType.X,
                    op=mybir.AluOpType.max)
                tt = g_pool.tile([128, E], F32, tag="tt")
                nc.vector.tensor_scalar(
                    tt[:], L[:], m1[:], None, op0=mybir.AluOpType.subtract)
                z = g_pool.tile([128, E], F32, tag="z")
                nc.vector.tensor_scalar(
                    z[:], tt[:], 0.0, None, op0=mybir.AluOpType.is_equal)
                msk = g_pool.tile([128, E], F32, tag="msk")
                nc.vector.scalar_tensor_tensor(
                    msk[:], z[:], -1e30, tt[:],
                    op0=mybir.AluOpType.mult, op1=mybir.AluOpType.add)
                m2 = g_pool.tile([128, 1], F32, tag="m2")
                nc.vector.tensor_reduce(
                    m2[:], msk[:], axis=mybir.AxisListType.X,
                    op=mybir.AluOpType.max)
                s2 = g_pool.tile([128, E], F32, tag="s2")
                nc.vector.tensor_scalar(
                    s2[:], tt[:], 2.0, m2[:],
                    op0=mybir.AluOpType.mult, op1=mybir.AluOpType.subtract)
                sg = g_pool.tile([128, E], F32, tag="sg")
                nc.scalar.activation(
                    sg[:], s2[:], mybir.ActivationFunctionType.Sigmoid)
                ge = g_pool.tile([128, E], F32, tag="ge")
                nc.vector.tensor_scalar(
                    ge[:], tt[:], m2[:], None, op0=mybir.AluOpType.is_ge)
                gates = g_pool.tile([128, E], F32, tag="gates")
                nc.vector.tensor_tensor(
                    gates[:], ge[:], sg[:], op=mybir.AluOpType.mult)
                gates_tl.append(gates)
            return gates_tl

        def emit_scale(g, mid_ps, gates_tl):
            """Scale mid rows by the per-expert gate (per-partition scalar),
            rounding to fp16."""
            mts_tm = []
            for tl in range(TPG):
                mt = mts_pool.tile([128, ER], F16, tag="mtm")
                for e in range(E):
                    nc.vector.tensor_scalar(
                        mt[:, e * R:(e + 1) * R],
                        mid_ps[:, tl, e * R:(e + 1) * R],
                        gates_tl[tl][:, e:e + 1], None,
                        op0=mybir.AluOpType.mult)
                mts_tm.append(mt)
            return mts_tm

        def emit_tail(g, mts_tm):
            """Transpose gated mid back to [er, t] (fp16), mm2, write out."""
            tok_g = g * GROUP
            mt_ps = ps_mt.tile([ER, TPG, 128], F16, tag="mt")
            mts_sb = mts_pool.tile([ER, TPG, 128], F16, tag="mts")
            for tl in range(TPG):
                nc.tensor.transpose(
                    mt_ps[:, tl, :], mts_tm[tl][:], id16_sb[:])
                copy_any(mts_sb[:, tl, :], mt_ps[:, tl, :])
            for tl in range(TPG):
                tok0 = tok_g + tl * 128
                dout_sb = dout_pool.tile([128, D], F16, tag="dout")
                for (d0, w) in MM2_CHUNKS:
                    mm2_ps = ps_mm2.tile([128, 512], F32, tag="mm2")
                    nc.tensor.matmul(
                        mm2_ps[:, 0:w],
                        mts_sb[:, tl, :],
                        b_sb[:, d0:d0 + w],
                    )
                    copy_any(dout_sb[:, d0:d0 + w], mm2_ps[:, 0:w])
                # stores go out via the (otherwise idle) Pool engine's SWDGE
                # path so they never head-of-line-block the SP load queue
                nc.gpsimd.dma_start(out_d[tok0:tok0 + 128, :], dout_sb[:])

        # Two-group software pipeline.  Per group, the PE stream is:
        # logits(g) -> transposes(g+2) -> mid(g) -> midT(g) -> mm2(g), so the
        # Pool-engine bf16 conversion for group g (issued two groups earlier)
        # and the gating chain for g are fully hidden behind transpose work.
        xt_a = emit_loads(0)
        emit_weights()
        xt_b = emit_loads(1)
        for g in range(N_GROUPS):
            cur = xt_a
            mid_ps = emit_logits(g, cur[0])
            gates_tl = emit_gates(g, mid_ps)
            if g + 2 < N_GROUPS:
                xt_a, xt_b = xt_b, emit_loads(g + 2)
            else:
                xt_a, xt_b = xt_b, None
            emit_mid(g, mid_ps, cur[1])
            mts_tm = emit_scale(g, mid_ps, gates_tl)
            emit_tail(g, mts_tm)


_CACHED = {}


def _build_module():
    key = "tm"
    if key in _CACHED:
        return _CACHED[key]
    nc = bacc.Bacc("TRN2", target_bir_lowering=False, debug=False)
    x_d = nc.dram_tensor("x_in", [T_C, D], F32, kind="ExternalInput").ap()
    wta_d = nc.dram_tensor(
        "wta_in", [128, D_CHUNKS, ER], BF16, kind="ExternalInput").ap()
    rw_d = nc.dram_tensor(
        "rw_in", [128, D_CHUNKS, E], F32, kind="ExternalInput").ap()
    b_d = nc.dram_tensor("ball_in", [ER, D], F16, kind="ExternalInput").ap()
    id_d = nc.dram_tensor("id_in", [128, 128], F32, kind="ExternalInput").ap()
    id16_d = nc.dram_tensor(
        "id16_in", [128, 128], F16, kind="ExternalInput").ap()
    out_d = nc.dram_tensor("out", [T_C, D], F16, kind="ExternalOutput").ap()
    with tile.TileContext(nc) as tc:
        build_kernel(tc, out_d, x_d, wta_d, rw_d, b_d, id_d, id16_d)
    nc.compile()
    _CACHED[key] = nc
    return nc


def _host_weights(router_w, A, B):
    import ml_dtypes
    # [128, D_CHUNKS, M]: element [p, c, m] = W.T[c*128 + p, m]
    WA = np.ascontiguousarray(
        A.reshape(ER, D).T.reshape(D_CHUNKS, 128, ER).transpose(1, 0, 2)
    ).astype(ml_dtypes.bfloat16)
    RW = np.ascontiguousarray(
        router_w.T.reshape(D_CHUNKS, 128, E).transpose(1, 0, 2)
    ).astype(np.float32)
    B_all = np.ascontiguousarray(
        (B.transpose(0, 2, 1).reshape(ER, D) * LORA_SCALE)).astype(np.float16)
    ident = np.eye(128, dtype=np.float32)
    ident16 = np.eye(128, dtype=np.float16)
    return WA, RW, B_all, ident, ident16


def make_in_maps(x, router_w, A, B):
    flat = np.ascontiguousarray(np.asarray(x, np.float32).reshape(T_FULL, D))
    WA, RW, B_all, ident, ident16 = _host_weights(
        np.asarray(router_w, np.float32),
        np.asarray(A, np.float32),
        np.asarray(B, np.float32))
    in_maps = []
    for i in range(N_CORES):
        in_maps.append({
            "x_in": flat[i * T_C:(i + 1) * T_C],
            "wta_in": WA,
            "rw_in": RW,
            "ball_in": B_all,
            "id_in": ident,
            "id16_in": ident16,
        })
    return in_maps


def kernel(x, router_w, A, B, _results_hook=None):
    from concourse.bass_utils import run_bass_kernel_spmd

    nc = _build_module()
    in_maps = make_in_maps(x, router_w, A, B)
    res = run_bass_kernel_spmd(nc, in_maps, core_ids=list(range(N_CORES)))
    if _results_hook is not None:
        _results_hook(res)
    out = np.concatenate(
        [np.asarray(res.results[i]["out"]) for i in range(N_CORES)], axis=0)
    return out.astype(np.float32).reshape(B_, S, D)


if __name__ == "__main__":
    rng = np.random.default_rng(0)
    x = rng.standard_normal((B_, S, D), dtype=np.float32)
    rw = (rng.standard_normal((E, D)) * 0.02).astype(np.float32)
    A = (rng.standard_normal((E, R, D)) * 0.02).astype(np.float32)
    Bm = (rng.standard_normal((E, D, R)) * 0.02).astype(np.float32)
    out = kernel(x, rw, A, Bm)
    print("out", out.shape, out.dtype, float(np.abs(out).max()))


# revision 48
# speedup vs baseline: 1.9417x; 1.0855x over previous
"""MoE LoRA delta kernel for Trainium2 (8 NeuronCores, data-parallel over tokens).

Computation (per token t):
    logits = x @ router_w.T                      [T, 4]
    gates  = top2-softmax(logits)                [T, 4]  (exactly 2 nonzero)
    mid    = x @ A_all.T                         [T, 64]   A_all[(e,r), d]
    delta  = (mid * expand(gates)) @ (4*B_all)   [T, D]    B_all[(e,r), d]

Kernel strategy per core (T_c = 1024 tokens, groups of 256):
  - x tiles are transposed on-chip (PE transpose, fp32 exact) into xt [D, T].
  - One fused fp32 matmul chain per 128-token tile computes token-major
    [128, 68] = x_tile @ [A_all.T | router_w.T]: cols 0:64 are mid, cols
    64:68 are the router logits (fp32-exact so the top-2 expert selection
    matches the fp32 reference).
  - Gating runs token-major in fp32: g_e = 1{t_e >= m2} * sigmoid(2*t_e - m2),
    t = l - max(l).  Gates scale mid in-place (per-expert per-partition
    scalar multiply) with fp16 output; the LoRA scale 4.0 is folded into B.
  - mid.T is restored with a tiny fp16 PE transpose, then mm2 (fp16) computes
    delta[t, d] = midT.T @ (4*B_all); results are written out as fp16 and
    upcast to fp32 on the host (max elementwise error ~1e-3 << 2e-2 gate).
"""

import os
import sys

for _p in ("/opt/trn_rl_repo", "/root/.axon_site/_ro/trn_rl_repo"):
    if os.path.isdir(_p) and _p not in sys.path:
        sys.path.insert(0, _p)

import numpy as np
from contextlib import ExitStack

import concourse.bass as bass
import concourse.bacc as bacc
import concourse.mybir as mybir
import concourse.tile as tile

N_CORES = 8
B_, S, D = 4, 2048, 3840
T_FULL = B_ * S                 # 8192
T_C = T_FULL // N_CORES         # 1024 tokens per core
E, R = 4, 16
ER = E * R                      # 64
M_W = ER + E                    # 68 = A rows + router rows
LORA_SCALE = 16.0 / np.sqrt(16.0)   # 4.0 (folded into B on the host)

GROUP = 256                     # tokens per group
TPG = GROUP // 128              # token tiles per group (2)
N_GROUPS = T_C // GROUP         # 4
D_CHUNKS = D // 128             # 30
TP_C = 3                        # chunks per transpose-psum tile
MM2_CHUNKS = [(i * 512, min(512, D - i * 512)) for i in range((D + 511) // 512)]

F32 = mybir.dt.float32
F16 = mybir.dt.float16
BF16 = mybir.dt.bfloat16

# Number of D-chunks whose mid-chain matmuls run in bf16 (Pool-engine
# converts those xt chunks to bf16; 1c/row vs fp32's 4c/row).  The rest
# stay fp32, emitted first so the Pool conversion latency hides behind them.
BF_C = int(os.environ.get("MOE_BFC", "15"))


def build_kernel(tc: tile.TileContext, out_d, x_d, wta16_d, wta32_d, rw_d,
                 b_d, id_d, id16_d):
    nc = tc.nc
    with ExitStack() as ctx:
        const_pool = ctx.enter_context(tc.tile_pool(name="const", bufs=1))
        xin_pool = ctx.enter_context(tc.tile_pool(name="xin", bufs=4))
        xt_pool = ctx.enter_context(tc.tile_pool(name="xt", bufs=3))
        xt16_pool = ctx.enter_context(tc.tile_pool(name="xt16", bufs=2))
        mts_pool = ctx.enter_context(tc.tile_pool(name="mts", bufs=2))
        g_pool = ctx.enter_context(tc.tile_pool(name="gate", bufs=2))
        dout_pool = ctx.enter_context(tc.tile_pool(name="dout", bufs=2))
        ps_tp = ctx.enter_context(
            tc.tile_pool(name="ps_tp", bufs=3, space=bass.MemorySpace.PSUM))
        ps_mid = ctx.enter_context(
            tc.tile_pool(name="ps_mid", bufs=2, space=bass.MemorySpace.PSUM))
        ps_mm2 = ctx.enter_context(
            tc.tile_pool(name="ps_mm2", bufs=3, space=bass.MemorySpace.PSUM))

        # ---- constants / weights ----
        # identities are loaded right after the first x piece (emit_ids);
        # the big weight loads follow group 0's x tiles (emit_weights), so
        # neither sits on the startup critical path.
        id_sb = const_pool.tile([128, 128], F32, tag="ident")
        id16_sb = const_pool.tile([128, 128], F16, tag="ident16")

        def emit_ids():
            nc.sync.dma_start(id_sb[:], id_d[:])
            nc.sync.dma_start(id16_sb[:], id16_d[:])
        wta16_sb = const_pool.tile([128, max(BF_C, 1), ER], BF16, tag="wta16")
        wta32_sb = const_pool.tile(
            [128, max(D_CHUNKS - BF_C, 1), ER], F32, tag="wta32")
        rw_sb = const_pool.tile([128, D_CHUNKS, E], F32, tag="rw")
        b_sb = const_pool.tile([ER, D], F16, tag="ball")

        def emit_weights():
            nc.sync.dma_start(rw_sb[:], rw_d[:])
            if BF_C > 0:
                nc.sync.dma_start(wta16_sb[:], wta16_d[:])
            if BF_C < D_CHUNKS:
                nc.sync.dma_start(wta32_sb[:], wta32_d[:])
            nc.sync.dma_start(b_sb[:], b_d[:])

        copy_engines = [nc.vector, nc.scalar]
        cp_state = {"i": 0}

        def copy_any(dst, src):
            eng = copy_engines[cp_state["i"] % 2]
            cp_state["i"] += 1
            if eng is nc.vector:
                eng.tensor_copy(dst, src)
            else:
                eng.copy(dst, src)

        HC = D_CHUNKS // 2          # chunks per x half-tile (15)

        def emit_loads(g, first=False):
            """DMA x half-tiles, PE-transpose, copy to SBUF xt (fp32); a
            Pool-engine bulk copy then rounds the bf16 chunks for mm1.  The
            very first piece is split small so transposes start early."""
            tok_g = g * GROUP
            xt_sb = xt_pool.tile([128, D_CHUNKS, GROUP], F32, tag="xt")
            xt16_sb = None
            if BF_C > 0:
                xt16_sb = xt16_pool.tile(
                    [128, BF_C, GROUP], BF16, tag="xt16")
            for h in range(2):
                for tl in range(TPG):
                    tok0 = tok_g + tl * 128
                    pieces = [(0, HC)]
                    if first and h == 0 and tl == 0:
                        pieces = [(0, 6), (6, HC)]
                    for (ca, cb) in pieces:
                        x_sb = xin_pool.tile(
                            [128, (cb - ca) * 128], F32, tag="xin")
                        nc.sync.dma_start(
                            x_sb[:],
                            x_d[tok0:tok0 + 128,
                                (h * HC + ca) * 128:(h * HC + cb) * 128])
                        if first and h == 0 and tl == 0 and ca == 0:
                            emit_ids()
                        for c0 in range(ca, cb, TP_C):
                            tp_ps = ps_tp.tile([128, TP_C, 128], F32, tag="tp")
                            for cc in range(TP_C):
                                c = c0 + cc
                                nc.tensor.transpose(
                                    tp_ps[:, cc, :],
                                    x_sb[:, (c - ca) * 128:(c - ca + 1) * 128],
                                    id_sb[:])
                            copy_any(
                                xt_sb[:, h * HC + c0:h * HC + c0 + TP_C,
                                      tl * 128:(tl + 1) * 128],
                                tp_ps[:])
            if BF_C > 0:
                nc.gpsimd.tensor_copy(
                    xt16_sb[:], xt_sb[:, 0:BF_C, :])
            return xt_sb, xt16_sb

        def emit_logits(g, xt_sb):
            """Exact fp32 router logits, token-major ([128, 4] per ttile)."""
            mid_ps = ps_mid.tile([128, TPG, M_W], F32, tag="mid")
            for tl in range(TPG):
                for c in range(D_CHUNKS):
                    nc.tensor.matmul(
                        mid_ps[:, tl, ER:M_W],
                        xt_sb[:, c, tl * 128:(tl + 1) * 128],
                        rw_sb[:, c, :],
                        start=(c == 0),
                        stop=(c == D_CHUNKS - 1),
                    )
            return mid_ps

        def emit_mid(g, mid_ps, xt_sb, xt16_sb):
            """mid = x_tile @ A.T (64 cols, token-major): fp32 chunks first
            (no Pool dependency), then the Pool-converted bf16 chunks."""
            for tl in range(TPG):
                tsl = slice(tl * 128, (tl + 1) * 128)
                order = [("f32", c) for c in range(BF_C, D_CHUNKS)] + \
                        [("b16", c) for c in range(BF_C)]
                for i, (kind, c) in enumerate(order):
                    if kind == "f32":
                        lhs, rhs = xt_sb[:, c, tsl], wta32_sb[:, c - BF_C, :]
                    else:
                        lhs, rhs = xt16_sb[:, c, tsl], wta16_sb[:, c, :]
                    nc.tensor.matmul(
                        mid_ps[:, tl, 0:ER], lhs, rhs,
                        start=(i == 0), stop=(i == len(order) - 1))

        def emit_gates(g, mid_ps):
            """Top-2 softmax gating, token-major."""
            gates_tl = []
            for tl in range(TPG):
                L = g_pool.tile([128, E], F32, tag="L")
                nc.vector.tensor_copy(L[:], mid_ps[:, tl, ER:M_W])
                m1 = g_pool.tile([128, 1], F32, tag="m1")
                nc.vector.tensor_reduce(
                    m1[:], L[:], axis=mybir.AxisListType.X,
                    op=mybir.AluOpType.max)
                tt = g_pool.tile([128, E], F32, tag="tt")
                nc.vector.tensor_scalar(
                    tt[:], L[:], m1[:], None, op0=mybir.AluOpType.subtract)
                z = g_pool.tile([128, E], F32, tag="z")
                nc.vector.tensor_scalar(
                    z[:], tt[:], 0.0, None, op0=mybir.AluOpType.is_equal)
                msk = g_pool.tile([128, E], F32, tag="msk")
                nc.vector.scalar_tensor_tensor(
                    msk[:], z[:], -1e30, tt[:],
                    op0=mybir.AluOpType.mult, op1=mybir.AluOpType.add)
                m2 = g_pool.tile([128, 1], F32, tag="m2")
                nc.vector.tensor_reduce(
                    m2[:], msk[:], axis=mybir.AxisListType.X,
                    op=mybir.AluOpType.max)
                s2 = g_pool.tile([128, E], F32, tag="s2")
                nc.vector.tensor_scalar(
                    s2[:], tt[:], 2.0, m2[:],
                    op0=mybir.AluOpType.mult, op1=mybir.AluOpType.subtract)
                sg = g_pool.tile([128, E], F32, tag="sg")
                nc.scalar.activation(
                    sg[:], s2[:], mybir.ActivationFunctionType.Sigmoid)
                ge = g_pool.tile([128, E], F32, tag="ge")
                nc.vector.tensor_scalar(
                    ge[:], tt[:], m2[:], None, op0=mybir.AluOpType.is_ge)
                gates = g_pool.tile([128, E], F32, tag="gates")
                nc.vector.tensor_tensor(
                    gates[:], ge[:], sg[:], op=mybir.AluOpType.mult)
                gates_tl.append(gates)
            return gates_tl

        def emit_scale(g, mid_ps, gates_tl):
            """Scale mid rows by the per-expert gate (per-partition scalar),
            rounding to fp16."""
            mts_tm = []
            for tl in range(TPG):
                mt = mts_pool.tile([128, ER], F16, tag="mtm")
                for e in range(E):
                    nc.vector.tensor_scalar(
                        mt[:, e * R:(e + 1) * R],
                        mid_ps[:, tl, e * R:(e + 1) * R],
                        gates_tl[tl][:, e:e + 1], None,
                        op0=mybir.AluOpType.mult)
                mts_tm.append(mt)
            return mts_tm

        def emit_tail(g, mts_tm):
            """Transpose gated mid back to [er, t] (fp16), mm2, write out."""
            tok_g = g * GROUP
            mt_ps = ps_tp.tile([ER, TPG, 128], F16, tag="tp")
            mts_sb = mts_pool.tile([ER, TPG, 128], F16, tag="mts")
            for tl in range(TPG):
                nc.tensor.transpose(
                    mt_ps[:, tl, :], mts_tm[tl][:], id16_sb[:])
                copy_any(mts_sb[:, tl, :], mt_ps[:, tl, :])
            last = (g == N_GROUPS - 1)
            for tl in range(TPG):
                tok0 = tok_g + tl * 128
                dout_sb = dout_pool.tile([128, D], F16, tag="dout")
                # stores go out via the (otherwise idle) Pool engine's SWDGE
                # path so they never head-of-line-block the SP load queue;
                # piecewise stores let DMA drain while later chunks' copies
                # are still in flight (finest on the very last tile).
                if last and tl == TPG - 1:
                    cuts = {1: (0, 1024), 3: (1024, 2048), 5: (2048, 3072)}
                    fin = (3072, D)
                else:
                    cuts = {3: (0, 2048)}
                    fin = (2048, D)
                for ci, (d0, w) in enumerate(MM2_CHUNKS):
                    mm2_ps = ps_mm2.tile([128, 512], F32, tag="mm2")
                    nc.tensor.matmul(
                        mm2_ps[:, 0:w],
                        mts_sb[:, tl, :],
                        b_sb[:, d0:d0 + w],
                    )
                    copy_any(dout_sb[:, d0:d0 + w], mm2_ps[:, 0:w])
                    if ci in cuts:
                        a, b = cuts[ci]
                        nc.gpsimd.dma_start(
                            out_d[tok0:tok0 + 128, a:b], dout_sb[:, a:b])
                nc.gpsimd.dma_start(
                    out_d[tok0:tok0 + 128, fin[0]:fin[1]],
                    dout_sb[:, fin[0]:fin[1]])

        # Two-group software pipeline.  Per group, the PE stream is:
        # logits(g) -> transposes(g+2) -> mid(g) -> midT(g) -> mm2(g), so the
        # Pool-engine bf16 conversion for group g (issued two groups earlier)
        # and the gating chain for g are fully hidden behind transpose work.
        xts = {0: emit_loads(0, first=True)}
        emit_weights()
        xts[1] = emit_loads(1)
        for g in range(N_GROUPS):
            cur = xts.pop(g)
            mid_ps = emit_logits(g, cur[0])
            gates_tl = emit_gates(g, mid_ps)
            if g + 2 < N_GROUPS:
                xts[g + 2] = emit_loads(g + 2)
            emit_mid(g, mid_ps, cur[0], cur[1])
            mts_tm = emit_scale(g, mid_ps, gates_tl)
            emit_tail(g, mts_tm)


_CACHED = {}


def _build_module():
    key = "tm"
    if key in _CACHED:
        return _CACHED[key]
    nc = bacc.Bacc("TRN2", target_bir_lowering=False, debug=False)
    x_d = nc.dram_tensor("x_in", [T_C, D], F32, kind="ExternalInput").ap()
    wta16_d = nc.dram_tensor(
        "wta16_in", [128, max(BF_C, 1), ER], BF16,
        kind="ExternalInput").ap()
    wta32_d = nc.dram_tensor(
        "wta32_in", [128, max(D_CHUNKS - BF_C, 1), ER], F32,
        kind="ExternalInput").ap()
    rw_d = nc.dram_tensor(
        "rw_in", [128, D_CHUNKS, E], F32, kind="ExternalInput").ap()
    b_d = nc.dram_tensor("ball_in", [ER, D], F16, kind="ExternalInput").ap()
    id_d = nc.dram_tensor("id_in", [128, 128], F32, kind="ExternalInput").ap()
    id16_d = nc.dram_tensor(
        "id16_in", [128, 128], F16, kind="ExternalInput").ap()
    out_d = nc.dram_tensor("out", [T_C, D], F16, kind="ExternalOutput").ap()
    with tile.TileContext(nc) as tc:
        build_kernel(tc, out_d, x_d, wta16_d, wta32_d, rw_d, b_d, id_d,
                     id16_d)
    nc.compile()
    _CACHED[key] = nc
    return nc


def _host_weights(router_w, A, B):
    import ml_dtypes
    # [128, D_CHUNKS, M]: element [p, c, m] = W.T[c*128 + p, m]
    WA = np.ascontiguousarray(
        A.reshape(ER, D).T.reshape(D_CHUNKS, 128, ER).transpose(1, 0, 2))
    WA16 = np.ascontiguousarray(
        WA[:, 0:max(BF_C, 1), :]).astype(ml_dtypes.bfloat16)
    if BF_C < D_CHUNKS:
        WA32 = np.ascontiguousarray(WA[:, BF_C:, :]).astype(np.float32)
    else:
        WA32 = np.zeros((128, 1, ER), np.float32)
    RW = np.ascontiguousarray(
        router_w.T.reshape(D_CHUNKS, 128, E).transpose(1, 0, 2)
    ).astype(np.float32)
    B_all = np.ascontiguousarray(
        (B.transpose(0, 2, 1).reshape(ER, D) * LORA_SCALE)).astype(np.float16)
    ident = np.eye(128, dtype=np.float32)
    ident16 = np.eye(128, dtype=np.float16)
    return WA16, WA32, RW, B_all, ident, ident16


def make_in_maps(x, router_w, A, B):
    flat = np.ascontiguousarray(np.asarray(x, np.float32).reshape(T_FULL, D))
    WA16, WA32, RW, B_all, ident, ident16 = _host_weights(
        np.asarray(router_w, np.float32),
        np.asarray(A, np.float32),
        np.asarray(B, np.float32))
    in_maps = []
    for i in range(N_CORES):
        in_maps.append({
            "x_in": flat[i * T_C:(i + 1) * T_C],
            "wta16_in": WA16,
            "wta32_in": WA32,
            "rw_in": RW,
            "ball_in": B_all,
            "id_in": ident,
            "id16_in": ident16,
        })
    return in_maps


def kernel(x, router_w, A, B, _results_hook=None):
    from concourse.bass_utils import run_bass_kernel_spmd

    nc = _build_module()
    in_maps = make_in_maps(x, router_w, A, B)
    res = run_bass_kernel_spmd(nc, in_maps, core_ids=list(range(N_CORES)))
    if _results_hook is not None:
        _results_hook(res)
    out = np.concatenate(
        [np.asarray(res.results[i]["out"]) for i in range(N_CORES)], axis=0)
    return out.astype(np.float32).reshape(B_, S, D)


if __name__ == "__main__":
    rng = np.random.default_rng(0)
    x = rng.standard_normal((B_, S, D), dtype=np.float32)
    rw = (rng.standard_normal((E, D)) * 0.02).astype(np.float32)
    A = (rng.standard_normal((E, R, D)) * 0.02).astype(np.float32)
    Bm = (rng.standard_normal((E, D, R)) * 0.02).astype(np.float32)
    out = kernel(x, rw, A, Bm)
    print("out", out.shape, out.dtype, float(np.abs(out).max()))


# revision 54
# speedup vs baseline: 2.1456x; 1.1050x over previous
"""MoE LoRA delta kernel for Trainium2 (8 NeuronCores, data-parallel over tokens).

Computation (per token t):
    logits = x @ router_w.T                      [T, 4]
    gates  = top2-softmax(logits)                [T, 4]  (exactly 2 nonzero)
    mid    = x @ A_all.T                         [T, 64]   A_all[(e,r), d]
    delta  = (mid * expand(gates)) @ (4*B_all)   [T, D]    B_all[(e,r), d]

Kernel strategy per core (T_c = 1024 tokens, groups of 256):
  - x tiles are transposed on-chip (PE transpose, fp32 exact) into xt [D, T].
  - One fused fp32 matmul chain per 128-token tile computes token-major
    [128, 68] = x_tile @ [A_all.T | router_w.T]: cols 0:64 are mid, cols
    64:68 are the router logits (fp32-exact so the top-2 expert selection
    matches the fp32 reference).
  - Gating runs token-major in fp32: g_e = 1{t_e >= m2} * sigmoid(2*t_e - m2),
    t = l - max(l).  Gates scale mid in-place (per-expert per-partition
    scalar multiply) with fp16 output; the LoRA scale 4.0 is folded into B.
  - mid.T is restored with a tiny fp16 PE transpose, then mm2 (fp16) computes
    delta[t, d] = midT.T @ (4*B_all); results are written out as fp16 and
    upcast to fp32 on the host (max elementwise error ~1e-3 << 2e-2 gate).
"""

import os
import sys

for _p in ("/opt/trn_rl_repo", "/root/.axon_site/_ro/trn_rl_repo"):
    if os.path.isdir(_p) and _p not in sys.path:
        sys.path.insert(0, _p)

import numpy as np
from contextlib import ExitStack

import concourse.bass as bass
import concourse.bacc as bacc
import concourse.mybir as mybir
import concourse.tile as tile

N_CORES = 8
B_, S, D = 4, 2048, 3840
T_FULL = B_ * S                 # 8192
T_C = T_FULL // N_CORES         # 1024 tokens per core
E, R = 4, 16
ER = E * R                      # 64
M_W = ER + E                    # 68 = A rows + router rows
LORA_SCALE = 16.0 / np.sqrt(16.0)   # 4.0 (folded into B on the host)

GROUP = int(os.environ.get("MOE_GROUP", "128"))  # tokens per group
TPG = GROUP // 128              # token tiles per group (2)
N_GROUPS = T_C // GROUP         # 4
D_CHUNKS = D // 128             # 30
TP_C = 3                        # chunks per transpose-psum tile
MM2_CHUNKS = [(i * 512, min(512, D - i * 512)) for i in range((D + 511) // 512)]

F32 = mybir.dt.float32
F16 = mybir.dt.float16
BF16 = mybir.dt.bfloat16

# Number of D-chunks whose mid-chain matmuls run in bf16 (Pool-engine
# converts those xt chunks to bf16; 1c/row vs fp32's 4c/row).  The rest
# stay fp32, emitted first so the Pool conversion latency hides behind them.
BF_C = int(os.environ.get("MOE_BFC", "21"))


def build_kernel(tc: tile.TileContext, out_d, x_d, wta16_d, wta32_d, rw_d,
                 b_d, id_d, id16_d):
    nc = tc.nc
    with ExitStack() as ctx:
        const_pool = ctx.enter_context(tc.tile_pool(name="const", bufs=1))
        xin_pool = ctx.enter_context(tc.tile_pool(name="xin", bufs=4))
        xt_pool = ctx.enter_context(tc.tile_pool(name="xt", bufs=int(os.environ.get("MOE_LA", "2")) + 1))
        xt16_pool = ctx.enter_context(tc.tile_pool(name="xt16", bufs=int(os.environ.get("MOE_LA", "2"))))
        mts_pool = ctx.enter_context(tc.tile_pool(name="mts", bufs=2))
        g_pool = ctx.enter_context(tc.tile_pool(name="gate", bufs=2))
        dout_pool = ctx.enter_context(tc.tile_pool(name="dout", bufs=2))
        ps_tp = ctx.enter_context(
            tc.tile_pool(name="ps_tp", bufs=3, space=bass.MemorySpace.PSUM))
        ps_mid = ctx.enter_context(
            tc.tile_pool(name="ps_mid", bufs=2, space=bass.MemorySpace.PSUM))
        ps_mm2 = ctx.enter_context(
            tc.tile_pool(name="ps_mm2", bufs=3, space=bass.MemorySpace.PSUM))

        # ---- constants / weights ----
        # identities are loaded right after the first x piece (emit_ids);
        # the big weight loads follow group 0's x tiles (emit_weights), so
        # neither sits on the startup critical path.
        id_sb = const_pool.tile([128, 128], F32, tag="ident")
        id16_sb = const_pool.tile([128, 128], F16, tag="ident16")

        def emit_ids():
            nc.sync.dma_start(id_sb[:], id_d[:])
            nc.sync.dma_start(id16_sb[:], id16_d[:])
        wta16_sb = const_pool.tile([128, max(BF_C, 1), ER], BF16, tag="wta16")
        wta32_sb = const_pool.tile(
            [128, max(D_CHUNKS - BF_C, 1), ER], F32, tag="wta32")
        rw_sb = const_pool.tile([128, D_CHUNKS, E], F32, tag="rw")
        b_sb = const_pool.tile([ER, D], F16, tag="ball")

        def emit_weights():
            nc.sync.dma_start(rw_sb[:], rw_d[:])
            if BF_C > 0:
                nc.sync.dma_start(wta16_sb[:], wta16_d[:])
            if BF_C < D_CHUNKS:
                nc.sync.dma_start(wta32_sb[:], wta32_d[:])
            nc.sync.dma_start(b_sb[:], b_d[:])

        # DVE also runs the gating chains, so the Activation engine takes a
        # slightly larger share of the PSUM->SBUF copies (5 of every 9).
        cp_pattern = [0, 1]
        cp_state = {"i": 0}

        def copy_any(dst, src):
            k = cp_pattern[cp_state["i"] % len(cp_pattern)]
            cp_state["i"] += 1
            if k == 0:
                nc.vector.tensor_copy(dst, src)
            else:
                nc.scalar.copy(dst, src)

        HC = D_CHUNKS // 2          # chunks per x half-tile (15)

        def emit_loads(g, first=False):
            """DMA x half-tiles, PE-transpose, copy to SBUF xt (fp32); a
            Pool-engine bulk copy then rounds the bf16 chunks for mm1.  The
            very first piece is split small so transposes start early."""
            tok_g = g * GROUP
            xt_sb = xt_pool.tile([128, D_CHUNKS, GROUP], F32, tag="xt")
            xt16_sb = None
            if BF_C > 0:
                xt16_sb = xt16_pool.tile(
                    [128, BF_C, GROUP], BF16, tag="xt16")
            for h in range(2):
                for tl in range(TPG):
                    tok0 = tok_g + tl * 128
                    pieces = [(0, HC)]
                    if first and h == 0 and tl == 0:
                        pieces = [(0, 3), (3, 9), (9, HC)]
                    for (ca, cb) in pieces:
                        x_sb = xin_pool.tile(
                            [128, (cb - ca) * 128], F32, tag="xin")
                        nc.sync.dma_start(
                            x_sb[:],
                            x_d[tok0:tok0 + 128,
                                (h * HC + ca) * 128:(h * HC + cb) * 128])
                        if first and h == 0 and tl == 0 and ca == 0:
                            emit_ids()
                        for c0 in range(ca, cb, TP_C):
                            tp_ps = ps_tp.tile([128, TP_C, 128], F32, tag="tp")
                            for cc in range(TP_C):
                                c = c0 + cc
                                nc.tensor.transpose(
                                    tp_ps[:, cc, :],
                                    x_sb[:, (c - ca) * 128:(c - ca + 1) * 128],
                                    id_sb[:])
                            copy_any(
                                xt_sb[:, h * HC + c0:h * HC + c0 + TP_C,
                                      tl * 128:(tl + 1) * 128],
                                tp_ps[:])
            if BF_C > 0:
                nc.gpsimd.tensor_copy(
                    xt16_sb[:], xt_sb[:, 0:BF_C, :])
            return xt_sb, xt16_sb

        def emit_logits(g, xt_sb):
            """Exact fp32 router logits, token-major ([128, 4] per ttile)."""
            mid_ps = ps_mid.tile([128, TPG, M_W], F32, tag="mid")
            for tl in range(TPG):
                for c in range(D_CHUNKS):
                    nc.tensor.matmul(
                        mid_ps[:, tl, ER:M_W],
                        xt_sb[:, c, tl * 128:(tl + 1) * 128],
                        rw_sb[:, c, :],
                        start=(c == 0),
                        stop=(c == D_CHUNKS - 1),
                    )
            return mid_ps

        def emit_mid(g, mid_ps, xt_sb, xt16_sb):
            """mid = x_tile @ A.T (64 cols, token-major): fp32 chunks first
            (no Pool dependency), then the Pool-converted bf16 chunks."""
            for tl in range(TPG):
                tsl = slice(tl * 128, (tl + 1) * 128)
                order = [("f32", c) for c in range(BF_C, D_CHUNKS)] + \
                        [("b16", c) for c in range(BF_C)]
                for i, (kind, c) in enumerate(order):
                    if kind == "f32":
                        lhs, rhs = xt_sb[:, c, tsl], wta32_sb[:, c - BF_C, :]
                    else:
                        lhs, rhs = xt16_sb[:, c, tsl], wta16_sb[:, c, :]
                    nc.tensor.matmul(
                        mid_ps[:, tl, 0:ER], lhs, rhs,
                        start=(i == 0), stop=(i == len(order) - 1))

        def emit_gates(g, mid_ps):
            """Top-2 softmax gating, token-major."""
            gates_tl = []
            for tl in range(TPG):
                L = g_pool.tile([128, E], F32, tag="L")
                nc.vector.tensor_copy(L[:], mid_ps[:, tl, ER:M_W])
                m1 = g_pool.tile([128, 1], F32, tag="m1")
                nc.vector.tensor_reduce(
                    m1[:], L[:], axis=mybir.AxisListType.X,
                    op=mybir.AluOpType.max)
                tt = g_pool.tile([128, E], F32, tag="tt")
                nc.vector.tensor_scalar(
                    tt[:], L[:], m1[:], None, op0=mybir.AluOpType.subtract)
                z = g_pool.tile([128, E], F32, tag="z")
                nc.vector.tensor_scalar(
                    z[:], tt[:], 0.0, None, op0=mybir.AluOpType.is_equal)
                msk = g_pool.tile([128, E], F32, tag="msk")
                nc.vector.scalar_tensor_tensor(
                    msk[:], z[:], -1e30, tt[:],
                    op0=mybir.AluOpType.mult, op1=mybir.AluOpType.add)
                m2 = g_pool.tile([128, 1], F32, tag="m2")
                nc.vector.tensor_reduce(
                    m2[:], msk[:], axis=mybir.AxisListType.X,
                    op=mybir.AluOpType.max)
                s2 = g_pool.tile([128, E], F32, tag="s2")
                nc.vector.tensor_scalar(
                    s2[:], tt[:], 2.0, m2[:],
                    op0=mybir.AluOpType.mult, op1=mybir.AluOpType.subtract)
                sg = g_pool.tile([128, E], F32, tag="sg")
                nc.scalar.activation(
                    sg[:], s2[:], mybir.ActivationFunctionType.Sigmoid)
                ge = g_pool.tile([128, E], F32, tag="ge")
                nc.vector.tensor_scalar(
                    ge[:], tt[:], m2[:], None, op0=mybir.AluOpType.is_ge)
                gates = g_pool.tile([128, E], F32, tag="gates")
                nc.vector.tensor_tensor(
                    gates[:], ge[:], sg[:], op=mybir.AluOpType.mult)
                gates_tl.append(gates)
            return gates_tl

        def emit_scale(g, mid_ps, gates_tl):
            """Scale mid rows by the per-expert gate (per-partition scalar),
            rounding to fp16."""
            mts_tm = []
            for tl in range(TPG):
                mt = mts_pool.tile([128, ER], F16, tag="mtm")
                for e in range(E):
                    nc.vector.tensor_scalar(
                        mt[:, e * R:(e + 1) * R],
                        mid_ps[:, tl, e * R:(e + 1) * R],
                        gates_tl[tl][:, e:e + 1], None,
                        op0=mybir.AluOpType.mult)
                mts_tm.append(mt)
            return mts_tm

        def emit_tail(g, mts_tm):
            """Transpose gated mid back to [er, t] (fp16), mm2, write out."""
            tok_g = g * GROUP
            mt_ps = ps_tp.tile([ER, TPG, 128], F16, tag="tp")
            mts_sb = mts_pool.tile([ER, TPG, 128], F16, tag="mts")
            for tl in range(TPG):
                nc.tensor.transpose(
                    mt_ps[:, tl, :], mts_tm[tl][:], id16_sb[:])
                copy_any(mts_sb[:, tl, :], mt_ps[:, tl, :])
            last = (g == N_GROUPS - 1)
            for tl in range(TPG):
                tok0 = tok_g + tl * 128
                dout_sb = dout_pool.tile([128, D], F16, tag="dout")
                # stores go out via the (otherwise idle) Pool engine's SWDGE
                # path so they never head-of-line-block the SP load queue;
                # piecewise stores let DMA drain while later chunks' copies
                # are still in flight (finest on the very last tile).
                if last and tl == TPG - 1:
                    cuts = {1: (0, 1024), 3: (1024, 2048), 5: (2048, 3072)}
                    fin = (3072, D)
                else:
                    cuts = {3: (0, 2048)}
                    fin = (2048, D)
                for ci, (d0, w) in enumerate(MM2_CHUNKS):
                    mm2_ps = ps_mm2.tile([128, 512], F32, tag="mm2")
                    nc.tensor.matmul(
                        mm2_ps[:, 0:w],
                        mts_sb[:, tl, :],
                        b_sb[:, d0:d0 + w],
                    )
                    copy_any(dout_sb[:, d0:d0 + w], mm2_ps[:, 0:w])
                    if ci in cuts:
                        a, b = cuts[ci]
                        seng = nc.sync if last else nc.gpsimd
                        seng.dma_start(
                            out_d[tok0:tok0 + 128, a:b], dout_sb[:, a:b])
                (nc.sync if last else nc.gpsimd).dma_start(
                    out_d[tok0:tok0 + 128, fin[0]:fin[1]],
                    dout_sb[:, fin[0]:fin[1]])

        # Two-group software pipeline.  Per group, the PE stream is:
        # logits(g) -> transposes(g+2) -> mid(g) -> midT(g) -> mm2(g), so the
        # Pool-engine bf16 conversion for group g (issued two groups earlier)
        # and the gating chain for g are fully hidden behind transpose work.
        LOOKAHEAD = int(os.environ.get("MOE_LA", "2"))
        xts = {0: emit_loads(0, first=True)}
        emit_weights()
        for k in range(1, LOOKAHEAD):
            xts[k] = emit_loads(k)
        for g in range(N_GROUPS):
            cur = xts.pop(g)
            mid_ps = emit_logits(g, cur[0])
            gates_tl = emit_gates(g, mid_ps)
            if g + LOOKAHEAD < N_GROUPS:
                xts[g + LOOKAHEAD] = emit_loads(g + LOOKAHEAD)
            emit_mid(g, mid_ps, cur[0], cur[1])
            mts_tm = emit_scale(g, mid_ps, gates_tl)
            emit_tail(g, mts_tm)


_CACHED = {}


def _build_module():
    key = "tm"
    if key in _CACHED:
        return _CACHED[key]
    nc = bacc.Bacc("TRN2", target_bir_lowering=False, debug=False)
    x_d = nc.dram_tensor("x_in", [T_C, D], F32, kind="ExternalInput").ap()
    wta16_d = nc.dram_tensor(
        "wta16_in", [128, max(BF_C, 1), ER], BF16,
        kind="ExternalInput").ap()
    wta32_d = nc.dram_tensor(
        "wta32_in", [128, max(D_CHUNKS - BF_C, 1), ER], F32,
        kind="ExternalInput").ap()
    rw_d = nc.dram_tensor(
        "rw_in", [128, D_CHUNKS, E], F32, kind="ExternalInput").ap()
    b_d = nc.dram_tensor("ball_in", [ER, D], F16, kind="ExternalInput").ap()
    id_d = nc.dram_tensor("id_in", [128, 128], F32, kind="ExternalInput").ap()
    id16_d = nc.dram_tensor(
        "id16_in", [128, 128], F16, kind="ExternalInput").ap()
    out_d = nc.dram_tensor("out", [T_C, D], F16, kind="ExternalOutput").ap()
    with tile.TileContext(nc) as tc:
        build_kernel(tc, out_d, x_d, wta16_d, wta32_d, rw_d, b_d, id_d,
                     id16_d)
    nc.compile()
    _CACHED[key] = nc
    return nc


def _host_weights(router_w, A, B):
    import ml_dtypes
    # [128, D_CHUNKS, M]: element [p, c, m] = W.T[c*128 + p, m]
    WA = np.ascontiguousarray(
        A.reshape(ER, D).T.reshape(D_CHUNKS, 128, ER).transpose(1, 0, 2))
    WA16 = np.ascontiguousarray(
        WA[:, 0:max(BF_C, 1), :]).astype(ml_dtypes.bfloat16)
    if BF_C < D_CHUNKS:
        WA32 = np.ascontiguousarray(WA[:, BF_C:, :]).astype(np.float32)
    else:
        WA32 = np.zeros((128, 1, ER), np.float32)
    RW = np.ascontiguousarray(
        router_w.T.reshape(D_CHUNKS, 128, E).transpose(1, 0, 2)
    ).astype(np.float32)
    B_all = np.ascontiguousarray(
        (B.transpose(0, 2, 1).reshape(ER, D) * LORA_SCALE)).astype(np.float16)
    ident = np.eye(128, dtype=np.float32)
    ident16 = np.eye(128, dtype=np.float16)
    return WA16, WA32, RW, B_all, ident, ident16


def make_in_maps(x, router_w, A, B):
    flat = np.ascontiguousarray(np.asarray(x, np.float32).reshape(T_FULL, D))
    WA16, WA32, RW, B_all, ident, ident16 = _host_weights(
        np.asarray(router_w, np.float32),
        np.asarray(A, np.float32),
        np.asarray(B, np.float32))
    in_maps = []
    for i in range(N_CORES):
        in_maps.append({
            "x_in": flat[i * T_C:(i + 1) * T_C],
            "wta16_in": WA16,
            "wta32_in": WA32,
            "rw_in": RW,
            "ball_in": B_all,
            "id_in": ident,
            "id16_in": ident16,
        })
    return in_maps


def kernel(x, router_w, A, B, _results_hook=None):
    from concourse.bass_utils import run_bass_kernel_spmd

    nc = _build_module()
    in_maps = make_in_maps(x, router_w, A, B)
    res = run_bass_kernel_spmd(nc, in_maps, core_ids=list(range(N_CORES)))
    if _results_hook is not None:
        _results_hook(res)
    out = np.concatenate(
        [np.asarray(res.results[i]["out"]) for i in range(N_CORES)], axis=0)
    return out.astype(np.float32).reshape(B_, S, D)


if __name__ == "__main__":
    rng = np.random.default_rng(0)
    x = rng.standard_normal((B_, S, D), dtype=np.float32)
    rw = (rng.standard_normal((E, D)) * 0.02).astype(np.float32)
    A = (rng.standard_normal((E, R, D)) * 0.02).astype(np.float32)
    Bm = (rng.standard_normal((E, D, R)) * 0.02).astype(np.float32)
    out = kernel(x, rw, A, Bm)
    print("out", out.shape, out.dtype, float(np.abs(out).max()))


# revision 58
# speedup vs baseline: 2.1509x; 1.0025x over previous
"""MoE LoRA delta kernel for Trainium2 (8 NeuronCores, data-parallel over tokens).

Computation (per token t):
    logits = x @ router_w.T                      [T, 4]
    gates  = top2-softmax(logits)                [T, 4]  (exactly 2 nonzero)
    mid    = x @ A_all.T                         [T, 64]   A_all[(e,r), d]
    delta  = (mid * expand(gates)) @ (4*B_all)   [T, D]    B_all[(e,r), d]

Kernel strategy per core (T_c = 1024 tokens, software-pipelined groups of
128 tokens, loads running two groups ahead of compute):
  - x tiles are transposed on-chip (PE transpose, fp32 exact) into xt [D, T];
    PSUM->SBUF copies alternate between the DVE and Activation engines.
  - Router logits are computed token-major by a fp32 chain per token tile
    ([128, 4], stationary = xt chunk, moving = router weights) so the top-2
    expert selection matches the fp32 reference exactly.
  - mid = x @ A.T ([128, 64] token-major) runs as a hybrid chain: the
    otherwise-idle GPSIMD engine rounds the first BF_C xt chunks to bf16
    (1 cycle/row matmuls) while the remaining chunks go straight from the
    fp32 xt (4 cycles/row), emitted first so the conversion latency hides.
  - Gating runs token-major in fp32: g_e = 1{t_e >= m2} * sigmoid(2*t_e - m2),
    t = l - max(l).  Gates scale mid in-place (per-expert per-partition
    scalar multiply) with fp16 output; the LoRA scale 4.0 is folded into B.
  - mid.T is restored with a tiny fp16 PE transpose, then mm2 (fp16) computes
    delta[t, d] = midT.T @ (4*B_all); results are written out as fp16 and
    upcast to fp32 on the host (max elementwise error ~2e-3 << 2e-2 gate).
  - Stores ride the Pool engine's SWDGE path (never blocking the SP load
    queue); the final groups store piecewise via SP so the drain overlaps
    the last copies.
"""

import os
import sys

for _p in ("/opt/trn_rl_repo", "/root/.axon_site/_ro/trn_rl_repo"):
    if os.path.isdir(_p) and _p not in sys.path:
        sys.path.insert(0, _p)

import numpy as np
from contextlib import ExitStack

import concourse.bass as bass
import concourse.bacc as bacc
import concourse.mybir as mybir
import concourse.tile as tile

N_CORES = 8
B_, S, D = 4, 2048, 3840
T_FULL = B_ * S                 # 8192
T_C = T_FULL // N_CORES         # 1024 tokens per core
E, R = 4, 16
ER = E * R                      # 64
M_W = ER + E                    # 68 = A rows + router rows
LORA_SCALE = 16.0 / np.sqrt(16.0)   # 4.0 (folded into B on the host)

GROUP = int(os.environ.get("MOE_GROUP", "128"))  # tokens per group
TPG = GROUP // 128              # token tiles per group (2)
N_GROUPS = T_C // GROUP         # 4
D_CHUNKS = D // 128             # 30
TP_C = 3                        # chunks per transpose-psum tile
MM2_CHUNKS = [(i * 512, min(512, D - i * 512)) for i in range((D + 511) // 512)]

F32 = mybir.dt.float32
F16 = mybir.dt.float16
BF16 = mybir.dt.bfloat16

# Number of D-chunks whose mid-chain matmuls run in bf16 (Pool-engine
# converts those xt chunks to bf16; 1c/row vs fp32's 4c/row).  The rest
# stay fp32, emitted first so the Pool conversion latency hides behind them.
BF_C = int(os.environ.get("MOE_BFC", "21"))


def build_kernel(tc: tile.TileContext, out_d, x_d, wta16_d, wta32_d, rw_d,
                 b_d, id_d, id16_d):
    nc = tc.nc
    with ExitStack() as ctx:
        const_pool = ctx.enter_context(tc.tile_pool(name="const", bufs=1))
        xin_pool = ctx.enter_context(tc.tile_pool(name="xin", bufs=4))
        xt_pool = ctx.enter_context(tc.tile_pool(name="xt", bufs=int(os.environ.get("MOE_LA", "2")) + 1))
        xt16_pool = ctx.enter_context(tc.tile_pool(name="xt16", bufs=int(os.environ.get("MOE_LA", "2"))))
        mts_pool = ctx.enter_context(tc.tile_pool(name="mts", bufs=2))
        g_pool = ctx.enter_context(tc.tile_pool(name="gate", bufs=2))
        dout_pool = ctx.enter_context(tc.tile_pool(name="dout", bufs=2))
        ps_tp = ctx.enter_context(
            tc.tile_pool(name="ps_tp", bufs=3, space=bass.MemorySpace.PSUM))
        ps_mid = ctx.enter_context(
            tc.tile_pool(name="ps_mid", bufs=int(os.environ.get("MOE_PSMID", "2")), space=bass.MemorySpace.PSUM))
        ps_mm2 = ctx.enter_context(
            tc.tile_pool(name="ps_mm2", bufs=int(os.environ.get("MOE_PSMM2", "3")), space=bass.MemorySpace.PSUM))

        # ---- constants / weights ----
        # identities are loaded right after the first x piece (emit_ids);
        # the big weight loads follow group 0's x tiles (emit_weights), so
        # neither sits on the startup critical path.
        id_sb = const_pool.tile([128, 128], F32, tag="ident")
        id16_sb = const_pool.tile([128, 128], F16, tag="ident16")

        def emit_ids():
            nc.sync.dma_start(id_sb[:], id_d[:])
            nc.sync.dma_start(id16_sb[:], id16_d[:])
        wta16_sb = const_pool.tile([128, max(BF_C, 1), ER], BF16, tag="wta16")
        wta32_sb = const_pool.tile(
            [128, max(D_CHUNKS - BF_C, 1), ER], F32, tag="wta32")
        rw_sb = const_pool.tile([128, D_CHUNKS, E], F32, tag="rw")
        b_sb = const_pool.tile([ER, D], F16, tag="ball")

        def emit_weights():
            nc.sync.dma_start(rw_sb[:], rw_d[:])
            if BF_C > 0:
                nc.sync.dma_start(wta16_sb[:], wta16_d[:])
            if BF_C < D_CHUNKS:
                nc.sync.dma_start(wta32_sb[:], wta32_d[:])
            nc.sync.dma_start(b_sb[:], b_d[:])

        # DVE also runs the gating chains, so the Activation engine takes a
        # slightly larger share of the PSUM->SBUF copies (5 of every 9).
        cp_pattern = [0, 1]
        cp_state = {"i": 0}

        def copy_any(dst, src):
            k = cp_pattern[cp_state["i"] % len(cp_pattern)]
            cp_state["i"] += 1
            if k == 0:
                nc.vector.tensor_copy(dst, src)
            else:
                nc.scalar.copy(dst, src)

        HC = D_CHUNKS // 2          # chunks per x half-tile (15)

        def emit_loads(g, first=False):
            """DMA x half-tiles, PE-transpose, copy to SBUF xt (fp32); a
            Pool-engine bulk copy then rounds the bf16 chunks for mm1.  The
            very first piece is split small so transposes start early."""
            tok_g = g * GROUP
            xt_sb = xt_pool.tile([128, D_CHUNKS, GROUP], F32, tag="xt")
            xt16_sb = None
            if BF_C > 0:
                xt16_sb = xt16_pool.tile(
                    [128, BF_C, GROUP], BF16, tag="xt16")
            for h in range(2):
                for tl in range(TPG):
                    tok0 = tok_g + tl * 128
                    pieces = [(0, HC)]
                    if first and h == 0 and tl == 0:
                        pieces = [(0, 3), (3, 9), (9, HC)]
                    for (ca, cb) in pieces:
                        x_sb = xin_pool.tile(
                            [128, (cb - ca) * 128], F32, tag="xin")
                        nc.sync.dma_start(
                            x_sb[:],
                            x_d[tok0:tok0 + 128,
                                (h * HC + ca) * 128:(h * HC + cb) * 128])
                        if first and h == 0 and tl == 0 and ca == 0:
                            emit_ids()
                        for c0 in range(ca, cb, TP_C):
                            tp_ps = ps_tp.tile([128, TP_C, 128], F32, tag="tp")
                            for cc in range(TP_C):
                                c = c0 + cc
                                nc.tensor.transpose(
                                    tp_ps[:, cc, :],
                                    x_sb[:, (c - ca) * 128:(c - ca + 1) * 128],
                                    id_sb[:])
                            copy_any(
                                xt_sb[:, h * HC + c0:h * HC + c0 + TP_C,
                                      tl * 128:(tl + 1) * 128],
                                tp_ps[:])
            if BF_C > 0:
                nc.gpsimd.tensor_copy(
                    xt16_sb[:], xt_sb[:, 0:BF_C, :])
            return xt_sb, xt16_sb

        def emit_logits(g, xt_sb):
            """Exact fp32 router logits, token-major ([128, 4] per ttile)."""
            mid_ps = ps_mid.tile([128, TPG, M_W], F32, tag="mid")
            for tl in range(TPG):
                for c in range(D_CHUNKS):
                    nc.tensor.matmul(
                        mid_ps[:, tl, ER:M_W],
                        xt_sb[:, c, tl * 128:(tl + 1) * 128],
                        rw_sb[:, c, :],
                        start=(c == 0),
                        stop=(c == D_CHUNKS - 1),
                    )
            return mid_ps

        def emit_mid(g, mid_ps, xt_sb, xt16_sb):
            """mid = x_tile @ A.T (64 cols, token-major): fp32 chunks first
            (no Pool dependency), then the Pool-converted bf16 chunks."""
            for tl in range(TPG):
                tsl = slice(tl * 128, (tl + 1) * 128)
                order = [("f32", c) for c in range(BF_C, D_CHUNKS)] + \
                        [("b16", c) for c in range(BF_C)]
                for i, (kind, c) in enumerate(order):
                    if kind == "f32":
                        lhs, rhs = xt_sb[:, c, tsl], wta32_sb[:, c - BF_C, :]
                    else:
                        lhs, rhs = xt16_sb[:, c, tsl], wta16_sb[:, c, :]
                    nc.tensor.matmul(
                        mid_ps[:, tl, 0:ER], lhs, rhs,
                        start=(i == 0), stop=(i == len(order) - 1))

        def emit_gates(g, mid_ps):
            """Top-2 softmax gating, token-major."""
            gates_tl = []
            for tl in range(TPG):
                L = g_pool.tile([128, E], F32, tag="L")
                nc.vector.tensor_copy(L[:], mid_ps[:, tl, ER:M_W])
                m1 = g_pool.tile([128, 1], F32, tag="m1")
                nc.vector.tensor_reduce(
                    m1[:], L[:], axis=mybir.AxisListType.X,
                    op=mybir.AluOpType.max)
                tt = g_pool.tile([128, E], F32, tag="tt")
                nc.vector.tensor_scalar(
                    tt[:], L[:], m1[:], None, op0=mybir.AluOpType.subtract)
                z = g_pool.tile([128, E], F32, tag="z")
                nc.vector.tensor_scalar(
                    z[:], tt[:], 0.0, None, op0=mybir.AluOpType.is_equal)
                msk = g_pool.tile([128, E], F32, tag="msk")
                nc.vector.scalar_tensor_tensor(
                    msk[:], z[:], -1e30, tt[:],
                    op0=mybir.AluOpType.mult, op1=mybir.AluOpType.add)
                m2 = g_pool.tile([128, 1], F32, tag="m2")
                nc.vector.tensor_reduce(
                    m2[:], msk[:], axis=mybir.AxisListType.X,
                    op=mybir.AluOpType.max)
                s2 = g_pool.tile([128, E], F32, tag="s2")
                nc.vector.tensor_scalar(
                    s2[:], tt[:], 2.0, m2[:],
                    op0=mybir.AluOpType.mult, op1=mybir.AluOpType.subtract)
                sg = g_pool.tile([128, E], F32, tag="sg")
                nc.scalar.activation(
                    sg[:], s2[:], mybir.ActivationFunctionType.Sigmoid)
                ge = g_pool.tile([128, E], F32, tag="ge")
                nc.vector.tensor_scalar(
                    ge[:], tt[:], m2[:], None, op0=mybir.AluOpType.is_ge)
                gates = g_pool.tile([128, E], F32, tag="gates")
                nc.vector.tensor_tensor(
                    gates[:], ge[:], sg[:], op=mybir.AluOpType.mult)
                gates_tl.append(gates)
            return gates_tl

        def emit_scale(g, mid_ps, gates_tl):
            """Scale mid rows by the per-expert gate (per-partition scalar),
            rounding to fp16."""
            mts_tm = []
            for tl in range(TPG):
                mt = mts_pool.tile([128, ER], F16, tag="mtm")
                for e in range(E):
                    nc.vector.tensor_scalar(
                        mt[:, e * R:(e + 1) * R],
                        mid_ps[:, tl, e * R:(e + 1) * R],
                        gates_tl[tl][:, e:e + 1], None,
                        op0=mybir.AluOpType.mult)
                mts_tm.append(mt)
            return mts_tm

        def emit_tail(g, mts_tm):
            """Transpose gated mid back to [er, t] (fp16), mm2, write out."""
            tok_g = g * GROUP
            mt_ps = ps_tp.tile([ER, TPG, 128], F16, tag="tp")
            mts_sb = mts_pool.tile([ER, TPG, 128], F16, tag="mts")
            for tl in range(TPG):
                nc.tensor.transpose(
                    mt_ps[:, tl, :], mts_tm[tl][:], id16_sb[:])
                copy_any(mts_sb[:, tl, :], mt_ps[:, tl, :])
            last = (g >= N_GROUPS - int(os.environ.get("MOE_LASTN", "2")))
            for tl in range(TPG):
                tok0 = tok_g + tl * 128
                dout_sb = dout_pool.tile([128, D], F16, tag="dout")
                # stores go out via the (otherwise idle) Pool engine's SWDGE
                # path so they never head-of-line-block the SP load queue;
                # piecewise stores let DMA drain while later chunks' copies
                # are still in flight (finest on the very last tile).
                if last and tl == TPG - 1:
                    cuts = {1: (0, 1024), 3: (1024, 2048), 5: (2048, 3072)}
                    fin = (3072, D)
                else:
                    cuts = {3: (0, 2048)}
                    fin = (2048, D)
                for ci, (d0, w) in enumerate(MM2_CHUNKS):
                    mm2_ps = ps_mm2.tile([128, 512], F32, tag="mm2")
                    nc.tensor.matmul(
                        mm2_ps[:, 0:w],
                        mts_sb[:, tl, :],
                        b_sb[:, d0:d0 + w],
                    )
                    copy_any(dout_sb[:, d0:d0 + w], mm2_ps[:, 0:w])
                    if ci in cuts:
                        a, b = cuts[ci]
                        seng = nc.sync if last else nc.gpsimd
                        seng.dma_start(
                            out_d[tok0:tok0 + 128, a:b], dout_sb[:, a:b])
                (nc.sync if last else nc.gpsimd).dma_start(
                    out_d[tok0:tok0 + 128, fin[0]:fin[1]],
                    dout_sb[:, fin[0]:fin[1]])

        # Two-group software pipeline.  Per group, the PE stream is:
        # logits(g) -> transposes(g+2) -> mid(g) -> midT(g) -> mm2(g), so the
        # Pool-engine bf16 conversion for group g (issued two groups earlier)
        # and the gating chain for g are fully hidden behind transpose work.
        LOOKAHEAD = int(os.environ.get("MOE_LA", "2"))
        xts = {0: emit_loads(0, first=True)}
        emit_weights()
        for k in range(1, LOOKAHEAD):
            xts[k] = emit_loads(k)
        for g in range(N_GROUPS):
            cur = xts.pop(g)
            mid_ps = emit_logits(g, cur[0])
            gates_tl = emit_gates(g, mid_ps)
            if g + LOOKAHEAD < N_GROUPS:
                xts[g + LOOKAHEAD] = emit_loads(g + LOOKAHEAD)
            emit_mid(g, mid_ps, cur[0], cur[1])
            mts_tm = emit_scale(g, mid_ps, gates_tl)
            emit_tail(g, mts_tm)


_CACHED = {}


def _build_module():
    key = "tm"
    if key in _CACHED:
        return _CACHED[key]
    nc = bacc.Bacc("TRN2", target_bir_lowering=False, debug=False)
    x_d = nc.dram_tensor("x_in", [T_C, D], F32, kind="ExternalInput").ap()
    wta16_d = nc.dram_tensor(
        "wta16_in", [128, max(BF_C, 1), ER], BF16,
        kind="ExternalInput").ap()
    wta32_d = nc.dram_tensor(
        "wta32_in", [128, max(D_CHUNKS - BF_C, 1), ER], F32,
        kind="ExternalInput").ap()
    rw_d = nc.dram_tensor(
        "rw_in", [128, D_CHUNKS, E], F32, kind="ExternalInput").ap()
    b_d = nc.dram_tensor("ball_in", [ER, D], F16, kind="ExternalInput").ap()
    id_d = nc.dram_tensor("id_in", [128, 128], F32, kind="ExternalInput").ap()
    id16_d = nc.dram_tensor(
        "id16_in", [128, 128], F16, kind="ExternalInput").ap()
    out_d = nc.dram_tensor("out", [T_C, D], F16, kind="ExternalOutput").ap()
    with tile.TileContext(nc) as tc:
        build_kernel(tc, out_d, x_d, wta16_d, wta32_d, rw_d, b_d, id_d,
                     id16_d)
    nc.compile()
    _CACHED[key] = nc
    return nc


def _host_weights(router_w, A, B):
    import ml_dtypes
    # [128, D_CHUNKS, M]: element [p, c, m] = W.T[c*128 + p, m]
    WA = np.ascontiguousarray(
        A.reshape(ER, D).T.reshape(D_CHUNKS, 128, ER).transpose(1, 0, 2))
    WA16 = np.ascontiguousarray(
        WA[:, 0:max(BF_C, 1), :]).astype(ml_dtypes.bfloat16)
    if BF_C < D_CHUNKS:
        WA32 = np.ascontiguousarray(WA[:, BF_C:, :]).astype(np.float32)
    else:
        WA32 = np.zeros((128, 1, ER), np.float32)
    RW = np.ascontiguousarray(
        router_w.T.reshape(D_CHUNKS, 128, E).transpose(1, 0, 2)
    ).astype(np.float32)
    B_all = np.ascontiguousarray(
        (B.transpose(0, 2, 1).reshape(ER, D) * LORA_SCALE)).astype(np.float16)
    ident = np.eye(128, dtype=np.float32)
    ident16 = np.eye(128, dtype=np.float16)
    return WA16, WA32, RW, B_all, ident, ident16


def make_in_maps(x, router_w, A, B):
    flat = np.ascontiguousarray(np.asarray(x, np.float32).reshape(T_FULL, D))
    WA16, WA32, RW, B_all, ident, ident16 = _host_weights(
        np.asarray(router_w, np.float32),
        np.asarray(A, np.float32),
        np.asarray(B, np.float32))
    in_maps = []
    for i in range(N_CORES):
        in_maps.append({
            "x_in": flat[i * T_C:(i + 1) * T_C],
            "wta16_in": WA16,
            "wta32_in": WA32,
            "rw_in": RW,
            "ball_in": B_all,
            "id_in": ident,
            "id16_in": ident16,
        })
    return in_maps


def kernel(x, router_w, A, B, _results_hook=None):
    from concourse.bass_utils import run_bass_kernel_spmd

    nc = _build_module()
    in_maps = make_in_maps(x, router_w, A, B)
    res = run_bass_kernel_spmd(nc, in_maps, core_ids=list(range(N_CORES)))
    if _results_hook is not None:
        _results_hook(res)
    out = np.concatenate(
        [np.asarray(res.results[i]["out"]) for i in range(N_CORES)], axis=0)
    return out.astype(np.float32).reshape(B_, S, D)


if __name__ == "__main__":
    rng = np.random.default_rng(0)
    x = rng.standard_normal((B_, S, D), dtype=np.float32)
    rw = (rng.standard_normal((E, D)) * 0.02).astype(np.float32)
    A = (rng.standard_normal((E, R, D)) * 0.02).astype(np.float32)
    Bm = (rng.standard_normal((E, D, R)) * 0.02).astype(np.float32)
    out = kernel(x, rw, A, Bm)
    print("out", out.shape, out.dtype, float(np.abs(out).max()))


# revision 64
# speedup vs baseline: 2.1594x; 1.0040x over previous
"""MoE LoRA delta kernel for Trainium2 (8 NeuronCores, data-parallel over tokens).

Computation (per token t):
    logits = x @ router_w.T                      [T, 4]
    gates  = top2-softmax(logits)                [T, 4]  (exactly 2 nonzero)
    mid    = x @ A_all.T                         [T, 64]   A_all[(e,r), d]
    delta  = (mid * expand(gates)) @ (4*B_all)   [T, D]    B_all[(e,r), d]

Kernel strategy per core (T_c = 1024 tokens, software-pipelined groups of
128 tokens, loads running two groups ahead of compute):
  - x tiles are transposed on-chip (PE transpose, fp32 exact) into xt [D, T];
    PSUM->SBUF copies alternate between the DVE and Activation engines.
  - Router logits are computed token-major by a fp32 chain per token tile
    ([128, 4], stationary = xt chunk, moving = router weights) so the top-2
    expert selection matches the fp32 reference exactly.
  - mid = x @ A.T ([128, 64] token-major) runs as a hybrid chain: the
    otherwise-idle GPSIMD engine rounds the first BF_C xt chunks to bf16
    (1 cycle/row matmuls) while the remaining chunks go straight from the
    fp32 xt (4 cycles/row), emitted first so the conversion latency hides.
  - Gating runs token-major in fp32: g_e = 1{t_e >= m2} * sigmoid(2*t_e - m2),
    t = l - max(l).  Gates scale mid in-place (per-expert per-partition
    scalar multiply) with fp16 output; the LoRA scale 4.0 is folded into B.
  - mid.T is restored with a tiny fp16 PE transpose, then mm2 (fp16) computes
    delta[t, d] = midT.T @ (4*B_all); results are written out as fp16 and
    upcast to fp32 on the host (max elementwise error ~2e-3 << 2e-2 gate).
  - Stores ride the Pool engine's SWDGE path (never blocking the SP load
    queue); the final groups store piecewise via SP so the drain overlaps
    the last copies.
"""

import os
import sys

for _p in ("/opt/trn_rl_repo", "/root/.axon_site/_ro/trn_rl_repo"):
    if os.path.isdir(_p) and _p not in sys.path:
        sys.path.insert(0, _p)

import numpy as np
from contextlib import ExitStack

import concourse.bass as bass
import concourse.bacc as bacc
import concourse.mybir as mybir
import concourse.tile as tile

N_CORES = 8
B_, S, D = 4, 2048, 3840
T_FULL = B_ * S                 # 8192
T_C = T_FULL // N_CORES         # 1024 tokens per core
E, R = 4, 16
ER = E * R                      # 64
M_W = ER + E                    # 68 = A rows + router rows
LORA_SCALE = 16.0 / np.sqrt(16.0)   # 4.0 (folded into B on the host)

GROUP = int(os.environ.get("MOE_GROUP", "128"))  # tokens per group
TPG = GROUP // 128              # token tiles per group (2)
N_GROUPS = T_C // GROUP         # 4
D_CHUNKS = D // 128             # 30
TP_C = 3                        # chunks per transpose-psum tile
MM2_CHUNKS = [(i * 512, min(512, D - i * 512)) for i in range((D + 511) // 512)]

F32 = mybir.dt.float32
F16 = mybir.dt.float16
BF16 = mybir.dt.bfloat16

# Number of D-chunks whose mid-chain matmuls run in bf16 (Pool-engine
# converts those xt chunks to bf16; 1c/row vs fp32's 4c/row).  The rest
# stay fp32, emitted first so the Pool conversion latency hides behind them.
BF_C = int(os.environ.get("MOE_BFC", "21"))


def build_kernel(tc: tile.TileContext, out_d, x_d, wta16_d, wta32_d,
                 wta32b_d, rw_d, b_d, id_d, id16_d):
    nc = tc.nc
    with ExitStack() as ctx:
        const_pool = ctx.enter_context(tc.tile_pool(name="const", bufs=1))
        xin_pool = ctx.enter_context(tc.tile_pool(name="xin", bufs=4))
        xt_pool = ctx.enter_context(tc.tile_pool(name="xt", bufs=int(os.environ.get("MOE_LA", "2")) + 1))
        xt16_pool = ctx.enter_context(tc.tile_pool(name="xt16", bufs=int(os.environ.get("MOE_LA", "2"))))
        mts_pool = ctx.enter_context(tc.tile_pool(name="mts", bufs=2))
        g_pool = ctx.enter_context(tc.tile_pool(name="gate", bufs=2))
        dout_pool = ctx.enter_context(tc.tile_pool(name="dout", bufs=int(os.environ.get("MOE_DOUTB", "4"))))
        ps_tp = ctx.enter_context(
            tc.tile_pool(name="ps_tp", bufs=3, space=bass.MemorySpace.PSUM))
        ps_mid = ctx.enter_context(
            tc.tile_pool(name="ps_mid", bufs=int(os.environ.get("MOE_PSMID", "2")), space=bass.MemorySpace.PSUM))
        ps_mm2 = ctx.enter_context(
            tc.tile_pool(name="ps_mm2", bufs=int(os.environ.get("MOE_PSMM2", "3")), space=bass.MemorySpace.PSUM))

        # ---- constants / weights ----
        # identities are loaded right after the first x piece (emit_ids);
        # the big weight loads follow group 0's x tiles (emit_weights), so
        # neither sits on the startup critical path.
        id_sb = const_pool.tile([128, 128], F32, tag="ident")
        id16_sb = const_pool.tile([128, 128], F16, tag="ident16")

        def emit_ids():
            nc.sync.dma_start(id_sb[:], id_d[:])
            nc.sync.dma_start(id16_sb[:], id16_d[:])
        wta16_sb = const_pool.tile([128, max(BF_C, 1), ER], BF16, tag="wta16")
        wta32_sb = const_pool.tile(
            [128, max(D_CHUNKS - BF_C, 1), ER], F32, tag="wta32")
        wta32b_sb = const_pool.tile([128, max(BF_C, 1), ER], F32, tag="wta32b")
        rw_sb = const_pool.tile([128, D_CHUNKS, E], F32, tag="rw")
        b_sb = const_pool.tile([ER, D], F16, tag="ball")

        def emit_weights():
            nc.sync.dma_start(rw_sb[:], rw_d[:])
            if BF_C > 0:
                nc.sync.dma_start(wta16_sb[:], wta16_d[:])
            if BF_C < D_CHUNKS:
                nc.sync.dma_start(wta32_sb[:], wta32_d[:])
            if NOBF_LAST > 0 and BF_C > 0:
                nc.sync.dma_start(wta32b_sb[:], wta32b_d[:])
            nc.sync.dma_start(b_sb[:], b_d[:])

        # DVE also runs the gating chains, so the Activation engine takes a
        # slightly larger share of the PSUM->SBUF copies (5 of every 9).
        cp_pattern = [0, 1]
        cp_state = {"i": 0}

        def copy_any(dst, src):
            k = cp_pattern[cp_state["i"] % len(cp_pattern)]
            cp_state["i"] += 1
            if k == 0:
                nc.vector.tensor_copy(dst, src)
            else:
                nc.scalar.copy(dst, src)

        HC = D_CHUNKS // 2          # chunks per x half-tile (15)

        NOBF_LAST = int(os.environ.get("MOE_NOBF", "0"))

        def g_bfc(g):
            return 0 if g >= N_GROUPS - NOBF_LAST else BF_C

        def emit_loads(g, first=False):
            """DMA x half-tiles, PE-transpose, copy to SBUF xt (fp32); a
            Pool-engine bulk copy then rounds the bf16 chunks for mm1.  The
            very first piece is split small so transposes start early."""
            tok_g = g * GROUP
            xt_sb = xt_pool.tile([128, D_CHUNKS, GROUP], F32, tag="xt")
            xt16_sb = None
            if g_bfc(g) > 0:
                xt16_sb = xt16_pool.tile(
                    [128, BF_C, GROUP], BF16, tag="xt16")
            for h in range(2):
                for tl in range(TPG):
                    tok0 = tok_g + tl * 128
                    pieces = [(0, HC)]
                    if first and h == 0 and tl == 0:
                        pieces = [(0, 3), (3, 9), (9, HC)]
                    for (ca, cb) in pieces:
                        x_sb = xin_pool.tile(
                            [128, (cb - ca) * 128], F32, tag="xin")
                        nc.sync.dma_start(
                            x_sb[:],
                            x_d[tok0:tok0 + 128,
                                (h * HC + ca) * 128:(h * HC + cb) * 128])
                        if first and h == 0 and tl == 0 and ca == 0:
                            emit_ids()
                        for c0 in range(ca, cb, TP_C):
                            tp_ps = ps_tp.tile([128, TP_C, 128], F32, tag="tp")
                            for cc in range(TP_C):
                                c = c0 + cc
                                nc.tensor.transpose(
                                    tp_ps[:, cc, :],
                                    x_sb[:, (c - ca) * 128:(c - ca + 1) * 128],
                                    id_sb[:])
                            copy_any(
                                xt_sb[:, h * HC + c0:h * HC + c0 + TP_C,
                                      tl * 128:(tl + 1) * 128],
                                tp_ps[:])
            if g_bfc(g) > 0:
                nc.gpsimd.tensor_copy(
                    xt16_sb[:], xt_sb[:, 0:BF_C, :])
            return xt_sb, xt16_sb

        def emit_logits(g, xt_sb):
            """Exact fp32 router logits, token-major ([128, 4] per ttile)."""
            mid_ps = ps_mid.tile([128, TPG, M_W], F32, tag="mid")
            for tl in range(TPG):
                for c in range(D_CHUNKS):
                    nc.tensor.matmul(
                        mid_ps[:, tl, ER:M_W],
                        xt_sb[:, c, tl * 128:(tl + 1) * 128],
                        rw_sb[:, c, :],
                        start=(c == 0),
                        stop=(c == D_CHUNKS - 1),
                    )
            return mid_ps

        def emit_mid(g, mid_ps, xt_sb, xt16_sb):
            """mid = x_tile @ A.T (64 cols, token-major): fp32 chunks first
            (no Pool dependency), then the Pool-converted bf16 chunks."""
            bfc = g_bfc(g)
            for tl in range(TPG):
                tsl = slice(tl * 128, (tl + 1) * 128)
                order = [("f32", c) for c in range(bfc, D_CHUNKS)] + \
                        [("b16", c) for c in range(bfc)]
                for i, (kind, c) in enumerate(order):
                    if kind == "f32":
                        if c >= BF_C:
                            rhs = wta32_sb[:, c - BF_C, :]
                        else:
                            rhs = wta32b_sb[:, c, :]
                        lhs = xt_sb[:, c, tsl]
                    else:
                        lhs, rhs = xt16_sb[:, c, tsl], wta16_sb[:, c, :]
                    nc.tensor.matmul(
                        mid_ps[:, tl, 0:ER], lhs, rhs,
                        start=(i == 0), stop=(i == len(order) - 1))

        def emit_gates(g, mid_ps):
            """Top-2 softmax gating, token-major."""
            gates_tl = []
            for tl in range(TPG):
                L = g_pool.tile([128, E], F32, tag="L")
                nc.vector.tensor_copy(L[:], mid_ps[:, tl, ER:M_W])
                m1 = g_pool.tile([128, 1], F32, tag="m1")
                nc.vector.tensor_reduce(
                    m1[:], L[:], axis=mybir.AxisListType.X,
                    op=mybir.AluOpType.max)
                tt = g_pool.tile([128, E], F32, tag="tt")
                nc.vector.tensor_scalar(
                    tt[:], L[:], m1[:], None, op0=mybir.AluOpType.subtract)
                z = g_pool.tile([128, E], F32, tag="z")
                nc.vector.tensor_scalar(
                    z[:], tt[:], 0.0, None, op0=mybir.AluOpType.is_equal)
                msk = g_pool.tile([128, E], F32, tag="msk")
                nc.vector.scalar_tensor_tensor(
                    msk[:], z[:], -1e30, tt[:],
                    op0=mybir.AluOpType.mult, op1=mybir.AluOpType.add)
                m2 = g_pool.tile([128, 1], F32, tag="m2")
                nc.vector.tensor_reduce(
                    m2[:], msk[:], axis=mybir.AxisListType.X,
                    op=mybir.AluOpType.max)
                s2 = g_pool.tile([128, E], F32, tag="s2")
                nc.vector.tensor_scalar(
                    s2[:], tt[:], 2.0, m2[:],
                    op0=mybir.AluOpType.mult, op1=mybir.AluOpType.subtract)
                sg = g_pool.tile([128, E], F32, tag="sg")
                nc.scalar.activation(
                    sg[:], s2[:], mybir.ActivationFunctionType.Sigmoid)
                ge = g_pool.tile([128, E], F32, tag="ge")
                nc.vector.tensor_scalar(
                    ge[:], tt[:], m2[:], None, op0=mybir.AluOpType.is_ge)
                gates = g_pool.tile([128, E], F32, tag="gates")
                nc.vector.tensor_tensor(
                    gates[:], ge[:], sg[:], op=mybir.AluOpType.mult)
                gates_tl.append(gates)
            return gates_tl

        def emit_scale(g, mid_ps, gates_tl):
            """Scale mid rows by the per-expert gate (per-partition scalar),
            rounding to fp16."""
            mts_tm = []
            for tl in range(TPG):
                mt = mts_pool.tile([128, ER], F16, tag="mtm")
                for e in range(E):
                    nc.vector.tensor_scalar(
                        mt[:, e * R:(e + 1) * R],
                        mid_ps[:, tl, e * R:(e + 1) * R],
                        gates_tl[tl][:, e:e + 1], None,
                        op0=mybir.AluOpType.mult)
                mts_tm.append(mt)
            return mts_tm

        pending_stores = {}

        def flush_store(g):
            if g in pending_stores:
                tok0, dout_sb = pending_stores.pop(g)
                nc.gpsimd.dma_start(
                    out_d[tok0:tok0 + 128, :], dout_sb[:])

        def emit_tail(g, mts_tm):
            """Transpose gated mid back to [er, t] (fp16), mm2, write out."""
            tok_g = g * GROUP
            mt_ps = ps_tp.tile([ER, TPG, 128], F16, tag="tp")
            mts_sb = mts_pool.tile([ER, TPG, 128], F16, tag="mts")
            for tl in range(TPG):
                nc.tensor.transpose(
                    mt_ps[:, tl, :], mts_tm[tl][:], id16_sb[:])
                copy_any(mts_sb[:, tl, :], mt_ps[:, tl, :])
            last = (g >= N_GROUPS - int(os.environ.get("MOE_LASTN", "2")))
            for tl in range(TPG):
                tok0 = tok_g + tl * 128
                dout_sb = dout_pool.tile([128, D], F16, tag="dout")
                # stores go out via the (otherwise idle) Pool engine's SWDGE
                # path so they never head-of-line-block the SP load queue;
                # piecewise stores let DMA drain while later chunks' copies
                # are still in flight (finest on the very last tile).
                if last and tl == TPG - 1:
                    cuts = {1: (0, 1024), 3: (1024, 2048), 5: (2048, 3072)}
                    fin = (3072, D)
                else:
                    cuts = {3: (0, 2048)}
                    fin = (2048, D)
                for ci, (d0, w) in enumerate(MM2_CHUNKS):
                    mm2_ps = ps_mm2.tile([128, 512], F32, tag="mm2")
                    nc.tensor.matmul(
                        mm2_ps[:, 0:w],
                        mts_sb[:, tl, :],
                        b_sb[:, d0:d0 + w],
                    )
                    copy_any(dout_sb[:, d0:d0 + w], mm2_ps[:, 0:w])
                    if last and ci in cuts:
                        a, b = cuts[ci]
                        nc.sync.dma_start(
                            out_d[tok0:tok0 + 128, a:b], dout_sb[:, a:b])
                if last:
                    nc.sync.dma_start(
                        out_d[tok0:tok0 + 128, fin[0]:fin[1]],
                        dout_sb[:, fin[0]:fin[1]])
                else:
                    # deferred: emitted STORE_DEFER groups later so loads
                    # win the DMA FIFO mid-run
                    pending_stores[g] = (tok0, dout_sb)

        # Two-group software pipeline.  Per group, the PE stream is:
        # logits(g) -> transposes(g+2) -> mid(g) -> midT(g) -> mm2(g), so the
        # Pool-engine bf16 conversion for group g (issued two groups earlier)
        # and the gating chain for g are fully hidden behind transpose work.
        LOOKAHEAD = int(os.environ.get("MOE_LA", "2"))
        xts = {0: emit_loads(0, first=True)}
        emit_weights()
        for k in range(1, LOOKAHEAD):
            xts[k] = emit_loads(k)
        STORE_DEFER = int(os.environ.get("MOE_SDEFER", "2"))
        for g in range(N_GROUPS):
            cur = xts.pop(g)
            mid_ps = emit_logits(g, cur[0])
            gates_tl = emit_gates(g, mid_ps)
            if g + LOOKAHEAD < N_GROUPS:
                xts[g + LOOKAHEAD] = emit_loads(g + LOOKAHEAD)
            emit_mid(g, mid_ps, cur[0], cur[1])
            mts_tm = emit_scale(g, mid_ps, gates_tl)
            flush_store(g - STORE_DEFER)
            emit_tail(g, mts_tm)
        for g in sorted(pending_stores):
            flush_store(g)


_CACHED = {}


def _build_module():
    key = "tm"
    if key in _CACHED:
        return _CACHED[key]
    nc = bacc.Bacc("TRN2", target_bir_lowering=False, debug=False)
    x_d = nc.dram_tensor("x_in", [T_C, D], F32, kind="ExternalInput").ap()
    wta16_d = nc.dram_tensor(
        "wta16_in", [128, max(BF_C, 1), ER], BF16,
        kind="ExternalInput").ap()
    wta32_d = nc.dram_tensor(
        "wta32_in", [128, max(D_CHUNKS - BF_C, 1), ER], F32,
        kind="ExternalInput").ap()
    wta32b_d = nc.dram_tensor(
        "wta32b_in", [128, max(BF_C, 1), ER], F32,
        kind="ExternalInput").ap()
    rw_d = nc.dram_tensor(
        "rw_in", [128, D_CHUNKS, E], F32, kind="ExternalInput").ap()
    b_d = nc.dram_tensor("ball_in", [ER, D], F16, kind="ExternalInput").ap()
    id_d = nc.dram_tensor("id_in", [128, 128], F32, kind="ExternalInput").ap()
    id16_d = nc.dram_tensor(
        "id16_in", [128, 128], F16, kind="ExternalInput").ap()
    out_d = nc.dram_tensor("out", [T_C, D], F16, kind="ExternalOutput").ap()
    with tile.TileContext(nc) as tc:
        build_kernel(tc, out_d, x_d, wta16_d, wta32_d, wta32b_d, rw_d, b_d,
                     id_d, id16_d)
    nc.compile()
    _CACHED[key] = nc
    return nc


def _host_weights(router_w, A, B):
    import ml_dtypes
    # [128, D_CHUNKS, M]: element [p, c, m] = W.T[c*128 + p, m]
    WA = np.ascontiguousarray(
        A.reshape(ER, D).T.reshape(D_CHUNKS, 128, ER).transpose(1, 0, 2))
    WA16 = np.ascontiguousarray(
        WA[:, 0:max(BF_C, 1), :]).astype(ml_dtypes.bfloat16)
    if BF_C < D_CHUNKS:
        WA32 = np.ascontiguousarray(WA[:, BF_C:, :]).astype(np.float32)
    else:
        WA32 = np.zeros((128, 1, ER), np.float32)
    WA32B = np.ascontiguousarray(WA[:, 0:max(BF_C, 1), :]).astype(np.float32)
    RW = np.ascontiguousarray(
        router_w.T.reshape(D_CHUNKS, 128, E).transpose(1, 0, 2)
    ).astype(np.float32)
    B_all = np.ascontiguousarray(
        (B.transpose(0, 2, 1).reshape(ER, D) * LORA_SCALE)).astype(np.float16)
    ident = np.eye(128, dtype=np.float32)
    ident16 = np.eye(128, dtype=np.float16)
    return WA16, WA32, WA32B, RW, B_all, ident, ident16


def make_in_maps(x, router_w, A, B):
    flat = np.ascontiguousarray(np.asarray(x, np.float32).reshape(T_FULL, D))
    WA16, WA32, WA32B, RW, B_all, ident, ident16 = _host_weights(
        np.asarray(router_w, np.float32),
        np.asarray(A, np.float32),
        np.asarray(B, np.float32))
    in_maps = []
    for i in range(N_CORES):
        in_maps.append({
            "x_in": flat[i * T_C:(i + 1) * T_C],
            "wta16_in": WA16,
            "wta32_in": WA32,
            "wta32b_in": WA32B,
            "rw_in": RW,
            "ball_in": B_all,
            "id_in": ident,
            "id16_in": ident16,
        })
    return in_maps


def kernel(x, router_w, A, B, _results_hook=None):
    from concourse.bass_utils import run_bass_kernel_spmd

    nc = _build_module()
    in_maps = make_in_maps(x, router_w, A, B)
    res = run_bass_kernel_spmd(nc, in_maps, core_ids=list(range(N_CORES)))
    if _results_hook is not None:
        _results_hook(res)
    out = np.concatenate(
        [np.asarray(res.results[i]["out"]) for i in range(N_CORES)], axis=0)
    return out.astype(np.float32).reshape(B_, S, D)


if __name__ == "__main__":
    rng = np.random.default_rng(0)
    x = rng.standard_normal((B_, S, D), dtype=np.float32)
    rw = (rng.standard_normal((E, D)) * 0.02).astype(np.float32)
    A = (rng.standard_normal((E, R, D)) * 0.02).astype(np.float32)
    Bm = (rng.standard_normal((E, D, R)) * 0.02).astype(np.float32)
    out = kernel(x, rw, A, Bm)
    print("out", out.shape, out.dtype, float(np.abs(out).max()))
